# revision 1
# baseline (speedup 1.0000x reference)
"""DiffVG-style circle renderer on 8 Trainium2 NeuronCores.

Strategy: shard the 1024x1024 image by rows (128 rows per core). Each core
composites only the circles whose vertical span intersects its row band
(sigmoid coverage is < 1.2e-7 beyond r+8 px of the edge), processing each
circle front-to-back on a 224-column window around its center:

    cov = sigmoid(2*r - 2*sqrt(dx^2 + dy^2))    (per pixel)
    w   = T * cov                                (T = transmittance, init 1)
    C  += w * (alpha * color);  T -= alpha * w   (premultiplied accumulation)

Final:  rgb = C, a = 1 - T  (identical to the sequential 'over' scan).

Engine split per circle:
  PE     d^2 = dy^2 (+) dx^2 outer-sum; two circles per matmul via a K=8
         block-diagonal layout (bf16 hi/lo split operands keep f32-level
         accuracy at bf16 speed)
  ACT    batched sqrt (phase 1, sqrt table); per-circle
         sigmoid(-2*d + 2r) (phase 2, sigmoid table); B-channel scale copy
  DVE    w = T*cov, T-MAC, R-MAC, G-MAC on dynamic 224-px windows,
         ordered [w_k, R_{k-1}, T_k, G_{k-1}] to pad same-engine RAW
         interlocks on the serial T chain
  GPSIMD B-channel accumulate
Explicit dep edges keep all sqrts before all sigmoids (one table switch).
"""

import sys

if "/opt/trn_rl_repo" not in sys.path:
    sys.path.insert(0, "/opt/trn_rl_repo")

import numpy as np
import ml_dtypes

import concourse.bass as bass
import concourse.bacc as bacc
import concourse.mybir as mybir
from concourse.tile import TileContext, add_dep_helper
from concourse import bass_utils

H = 1024
W = 1024
ROWS = 128          # rows per core
N_CORES = 8
WIN = 224           # column window per circle (covers 2*(r+8) for r < 100)
MARGIN = 8.0        # sigmoid(-2*8) ~ 1.1e-7
CHUNK = 72          # max circle slots per phase pass (SBUF budget)
F32 = mybir.dt.float32
BF16 = mybir.dt.bfloat16
AF = mybir.ActivationFunctionType
OP = mybir.AluOpType
BF = ml_dtypes.bfloat16


def _build_core_inputs(centers, radii, colors, core):
    """Per-core circle list (slots ordered top-circle-first)."""
    y0 = ROWS * core
    cy = centers[:, 1].astype(np.float64)
    cx = centers[:, 0].astype(np.float64)
    r = radii.astype(np.float64)
    keep = (cy + r + MARGIN >= y0 + 0.5) & (cy - r - MARGIN <= y0 + ROWS - 0.5)
    idx = np.where(keep)[0][::-1]  # reversed: topmost (last-drawn) first
    return idx, cx[idx], cy[idx], r[idx], colors[idx].astype(np.float64)


def _hilo(x):
    hi = x.astype(BF)
    lo = (x - hi.astype(np.float64)).astype(BF)
    return hi, lo


def make_inputs(centers, radii, colors, nc_slots):
    assert nc_slots % 8 == 0
    ins = []
    for core in range(N_CORES):
        y0 = ROWS * core
        idx, cx, cy, r, col = _build_core_inputs(centers, radii, colors, core)
        n = len(idx)
        assert n <= nc_slots
        scal = np.zeros((ROWS, nc_slots * 8), np.float32)
        offs = np.zeros((1, nc_slots), np.int32)
        # two circles (a, b) share one K=8 matmul: lhsT rows 0-3 belong to
        # a, rows 4-7 to b; rhs zero-masks the other circle's columns.
        lhsT = np.zeros((8, (nc_slots // 2) * ROWS), BF)
        rhs = np.zeros((8, nc_slots * WIN), BF)

        p = np.arange(ROWS, dtype=np.float64)
        j = np.arange(WIN, dtype=np.float64)
        for k in range(n):
            off = int(np.clip(np.floor(cx[k]) - 112.0, 0.0, float(W - WIN)))
            offs[0, k] = off
            dy2 = (y0 + p + 0.5 - cy[k]) ** 2
            dx2 = (off + j + 0.5 - cx[k]) ** 2
            alpha = col[k, 3]
            scal[:, k * 8 + 2] = 2.0 * r[k]
            scal[:, k * 8 + 3] = alpha * col[k, 0]
            scal[:, k * 8 + 4] = alpha * col[k, 1]
            scal[:, k * 8 + 5] = alpha * col[k, 2]
            scal[:, k * 8 + 6] = -alpha
            yh, yl = _hilo(dy2)
            xh, xl = _hilo(dx2)
            pair, half = divmod(k, 2)
            rbase = 4 * half
            ls = slice(pair * ROWS, (pair + 1) * ROWS)
            lhsT[rbase + 0, ls] = yh
            lhsT[rbase + 1, ls] = yl
            lhsT[rbase + 2, ls] = 1.0
            lhsT[rbase + 3, ls] = 1.0
            rs = slice(k * WIN, (k + 1) * WIN)
            rhs[rbase + 0, rs] = 1.0
            rhs[rbase + 1, rs] = 1.0
            rhs[rbase + 2, rs] = xh
            rhs[rbase + 3, rs] = xl
        ins.append({"scal": scal, "offs": offs, "lhsT": lhsT, "rhs": rhs})
    return ins


def build_nc(nc_slots):
    assert nc_slots % 8 == 0
    nc = bacc.Bacc("TRN2", target_bir_lowering=False, debug=False,
                   num_devices=N_CORES)
    scal_d = nc.dram_tensor("scal", [ROWS, nc_slots * 8], F32,
                            kind="ExternalInput").ap()
    offs_d = nc.dram_tensor("offs", [1, nc_slots], mybir.dt.int32,
                            kind="ExternalInput").ap()
    lhsT_d = nc.dram_tensor("lhsT", [8, (nc_slots // 2) * ROWS], BF16,
                            kind="ExternalInput").ap()
    rhs_d = nc.dram_tensor("rhs", [8, nc_slots * WIN], BF16,
                           kind="ExternalInput").ap()
    out_d = nc.dram_tensor("out", [ROWS, W * 4], F32,
                           kind="ExternalOutput").ap()

    with TileContext(nc) as tc:
        # persistent state
        T = nc.alloc_sbuf_tensor("T", [ROWS, W], F32).ap()
        CR = nc.alloc_sbuf_tensor("CR", [ROWS, W], F32).ap()
        CG = nc.alloc_sbuf_tensor("CG", [ROWS, W], F32).ap()
        CB = nc.alloc_sbuf_tensor("CB", [ROWS, W], F32).ap()
        out_sb = nc.alloc_sbuf_tensor("out_sb", [ROWS, W * 4], F32).ap()
        ch = min(CHUNK, nc_slots)
        dring = nc.alloc_sbuf_tensor("dring", [ROWS, ch * WIN], F32).ap()
        scal_sb = nc.alloc_sbuf_tensor("scal_sb", [ROWS, nc_slots * 8],
                                       F32).ap()
        offs_sb = nc.alloc_sbuf_tensor("offs_sb", [1, nc_slots],
                                       mybir.dt.int32).ap()

        nc.sync.dma_start(scal_sb, scal_d)
        nc.sync.dma_start(offs_sb, offs_d)
        nc.vector.memset(T, 1.0)
        nc.vector.memset(CR, 0.0)
        nc.gpsimd.memset(CG, 0.0)
        nc.gpsimd.memset(CB, 0.0)

        with (
            tc.tile_pool(name="psum", bufs=2, space="PSUM") as psum_pool,
            tc.tile_pool(name="ops", bufs=3) as oppool,
            tc.tile_pool(name="cov", bufs=6) as covpool,
            tc.tile_pool(name="w", bufs=6) as wpool,
            tc.tile_pool(name="tmpb", bufs=6) as bpool,
        ):
            prev_v = None
            prev_g = None
            # pending R/G MAC for the previous circle (emitted one circle
            # late so the serial T chain never reads a value written by
            # the immediately preceding DVE instruction)
            pend = None

            for chunk0 in range(0, nc_slots, CHUNK):
                nk = min(CHUNK, nc_slots - chunk0)
                assert nk % 8 == 0
                # ---------- phase 1: d2 (PE, paired) -> batched sqrt ----
                sqrt_instrs = []
                for g8 in range(0, nk, 8):
                    lh_t = oppool.tile([8, 4 * ROWS], BF16, tag="lh")
                    rh_t = oppool.tile([8, 8 * WIN], BF16, tag="rh")
                    k0 = chunk0 + g8
                    p0 = k0 // 2
                    nc.sync.dma_start(
                        lh_t, lhsT_d[:, p0 * ROWS:(p0 + 4) * ROWS])
                    nc.sync.dma_start(
                        rh_t, rhs_d[:, k0 * WIN:(k0 + 8) * WIN])
                    pt = psum_pool.tile([ROWS, 4 * 512], F32)
                    for i in range(4):
                        nc.tensor.matmul(
                            pt[:, i * 512:i * 512 + 2 * WIN],
                            lh_t[:, i * ROWS:(i + 1) * ROWS],
                            rh_t[:, i * 2 * WIN:(i + 1) * 2 * WIN],
                            start=True, stop=True)
                    pview = pt.rearrange("p (b f) -> p b f", f=512)
                    dbase = g8 * WIN
                    dview = dring[:, dbase:dbase + 8 * WIN].rearrange(
                        "p (b f) -> p b f", f=2 * WIN)
                    si = nc.scalar.activation(
                        dview, pview[:, :, :2 * WIN], AF.Sqrt)
                    if sqrt_instrs:
                        add_dep_helper(si.ins, sqrt_instrs[-1].ins,
                                       sync=False, reason="ACT table order")
                    sqrt_instrs.append(si)

                # ---------- phase 2: sigmoid (ACT) -> composite (DVE/GPS) ---
                for g8 in range(0, nk, 8):
                    k0 = chunk0 + g8
                    vregs = [nc.vector.alloc_register(f"offv_{k0}_{i}")
                             for i in range(8)]
                    liv = nc.vector.reg_load(vregs, offs_sb[0:1, k0:k0 + 8])
                    if prev_v is not None:
                        add_dep_helper(liv.ins, prev_v.ins, sync=False,
                                       reason="reg pressure")
                    voff = [nc.vector.snap(rg, donate=True,
                                           min_val=0, max_val=W - WIN)
                            for rg in vregs]
                    gregs = [nc.gpsimd.alloc_register(f"offg_{k0}_{i}")
                             for i in range(8)]
                    lig = nc.gpsimd.reg_load(gregs, offs_sb[0:1, k0:k0 + 8])
                    if prev_g is not None:
                        add_dep_helper(lig.ins, prev_g.ins, sync=False,
                                       reason="reg pressure")
                    goff = [nc.gpsimd.snap(rg, donate=True,
                                           min_val=0, max_val=W - WIN)
                            for rg in gregs]
                    for i in range(8):
                        k = chunk0 + g8 + i
                        kl = k - chunk0
                        cov = covpool.tile([ROWS, WIN], F32)
                        sg = nc.scalar.activation(
                            cov, dring[:, kl * WIN:(kl + 1) * WIN],
                            AF.Sigmoid,
                            bias=scal_sb[:, k * 8 + 2:k * 8 + 3],
                            scale=-2.0)
                        add_dep_helper(sg.ins, sqrt_instrs[-1].ins,
                                       sync=False,
                                       reason="sigmoid after all sqrt")
                        # DVE order: w_k, R_{k-1}, T_k, G_{k-1}
                        tw = T[:, bass.ds(voff[i], WIN)]
                        w = wpool.tile([ROWS, WIN], F32)
                        nc.vector.tensor_tensor(w, tw, cov, OP.mult)
                        tmpb = bpool.tile([ROWS, WIN], F32)
                        nc.scalar.activation(
                            tmpb, w, AF.Copy,
                            scale=scal_sb[:, k * 8 + 5:k * 8 + 6])
                        cbw = CB[:, bass.ds(goff[i], WIN)]
                        prev_g = nc.gpsimd.tensor_tensor(
                            cbw, cbw, tmpb, OP.add)
                        wp, kp, offp = (pend if pend is not None
                                        else (None, None, None))
                        if wp is not None:
                            crw = CR[:, bass.ds(offp, WIN)]
                            nc.vector.scalar_tensor_tensor(
                                crw, wp, scal_sb[:, kp * 8 + 3:kp * 8 + 4],
                                crw, OP.mult, OP.add)
                        nc.vector.scalar_tensor_tensor(
                            tw, w, scal_sb[:, k * 8 + 6:k * 8 + 7], tw,
                            OP.mult, OP.add)
                        if wp is not None:
                            cgw = CG[:, bass.ds(offp, WIN)]
                            prev_v = nc.vector.scalar_tensor_tensor(
                                cgw, wp, scal_sb[:, kp * 8 + 4:kp * 8 + 5],
                                cgw, OP.mult, OP.add)
                        pend = (w, k, voff[i])

            # flush the last circle's R/G MACs
            if pend is not None:
                wp, kp, offp = pend
                crw = CR[:, bass.ds(offp, WIN)]
                nc.vector.scalar_tensor_tensor(
                    crw, wp, scal_sb[:, kp * 8 + 3:kp * 8 + 4], crw,
                    OP.mult, OP.add)
                cgw = CG[:, bass.ds(offp, WIN)]
                nc.vector.scalar_tensor_tensor(
                    cgw, wp, scal_sb[:, kp * 8 + 4:kp * 8 + 5], cgw,
                    OP.mult, OP.add)
                pend = None

        # ---------- finish: interleave RGBA and store (2 halves) ----------
        ov = out_sb.rearrange("p (x c) -> p x c", c=4)
        HW4 = W // 4
        for hx in range(4):
            cs = slice(hx * HW4, (hx + 1) * HW4)
            nc.vector.tensor_copy(ov[:, cs, 0], CR[:, cs])
            nc.scalar.activation(ov[:, cs, 1], CG[:, cs], AF.Copy)
            nc.gpsimd.tensor_copy(ov[:, cs, 2], CB[:, cs])
            nc.vector.tensor_scalar(ov[:, cs, 3], T[:, cs], -1.0, 1.0,
                                    OP.mult, OP.add)
            nc.sync.dma_start(out_d[:, hx * 4 * HW4:(hx + 1) * 4 * HW4],
                              out_sb[:, hx * 4 * HW4:(hx + 1) * 4 * HW4])

    nc.compile()
    return nc


_CACHE = {}


def _get_nc(nc_slots):
    if nc_slots not in _CACHE:
        _CACHE[nc_slots] = build_nc(nc_slots)
    return _CACHE[nc_slots]


def kernel(centers, radii, colors):
    centers = np.asarray(centers, np.float32)
    radii = np.asarray(radii, np.float32)
    colors = np.asarray(colors, np.float32)

    counts = []
    for core in range(N_CORES):
        idx, *_ = _build_core_inputs(centers, radii, colors, core)
        counts.append(len(idx))
    nc_slots = max(8, ((max(counts) + 7) // 8) * 8)

    nc = _get_nc(nc_slots)
    ins = make_inputs(centers, radii, colors, nc_slots)
    res = bass_utils.run_bass_kernel_spmd(nc, ins, list(range(N_CORES)),
                                          trace=False)
    out = np.concatenate(
        [res.results[c]["out"].reshape(ROWS, W, 4) for c in range(N_CORES)],
        axis=0)
    return out



# revision 3
# speedup vs baseline: 1.0282x; 1.0282x over previous
"""DiffVG-style circle renderer on 8 Trainium2 NeuronCores.

Strategy: shard the 1024x1024 image by rows (128 rows per core). Each core
composites the circles whose vertical span intersects its row band,
front-to-back with transmittance T:

    cov = sigmoid(r - d^2/r)          ~= sigmoid(2(r - d)) near the edge
    w   = T * cov                      (w ring, fp16)
    T  *= (1 - a*cov)                  (mtau ring premultiplied on DVE)
    C_ch += (a*col_ch) * w             (premultiplied m_ch on DVE)

Front-to-back order is relaxed: circles whose column windows don't overlap
commute, so each core emits a width-descending order compatible with the
z partial order. Slot k's window width is the max over cores of the k-th
emitted circle width (compile-time constant); offsets are runtime data.

Engine split (all dynamic-window ops as cheap Pool tensor_tensor):
  PE     z = (r^2 - d^2)/r outer-sum; two circles per K=8 matmul
         (bf16 hi/lo split operands), bias folded in -> no sqrt pass
  ACT    per-pair sigmoid PSUM -> fp16 cov ring
  DVE    static-ring tensor_scalar premultiplies: mtau = 1 - a*cov,
         m_ch = (a col_ch) * w
  Pool   w = T*cov, T *= mtau, C_ch += m_ch  on dynamic windows
State T, CR, CG, CB are fp16 planes; output = 4 fp16 planes DMA'd out,
assembled/converted to f32 on host.
"""

import sys

if "/opt/trn_rl_repo" not in sys.path:
    sys.path.insert(0, "/opt/trn_rl_repo")

import numpy as np
import ml_dtypes

import concourse.bass as bass
import concourse.bacc as bacc
import concourse.mybir as mybir
from concourse.tile import TileContext
from concourse import bass_utils

H = 1024
W = 1024
ROWS = 128
N_CORES = 8
MARGIN = 6.0
ROUND = 16
WMIN = 32
WCAP = 224
F32 = mybir.dt.float32
F16 = mybir.dt.float16
BF16 = mybir.dt.bfloat16
I32 = mybir.dt.int32
AF = mybir.ActivationFunctionType
OP = mybir.AluOpType
BF = ml_dtypes.bfloat16


# ---------------------------------------------------------------- host plan
def _core_circles(centers, radii, core):
    """Kept circle indices + cap-clipped rounded widths + offsets."""
    y0 = ROWS * core
    cy = centers[:, 1].astype(np.float64)
    cx = centers[:, 0].astype(np.float64)
    r = radii.astype(np.float64)
    keep = (cy + r + MARGIN >= y0 + 0.5) & (cy - r - MARGIN <= y0 + ROWS - 0.5)
    idx = np.where(keep)[0]
    dymin = np.maximum(0.0, np.maximum(y0 + 0.5 - cy[idx],
                                       cy[idx] - (y0 + ROWS - 0.5)))
    rm = r[idx] + MARGIN
    halfw = np.sqrt(np.maximum(rm * rm - dymin * dymin, 4.0))
    ws = np.clip(np.ceil(2.0 * halfw / ROUND) * ROUND, WMIN, WCAP).astype(int)
    off = np.clip(np.round(cx[idx] - ws / 2.0), 0, W - ws).astype(int)
    return idx, ws, off


def _greedy_f2b(idx, ws, off):
    """Front-to-back (topmost first) order, widest-available-first among
    circles whose later-drawn column-overlapping circles are all emitted."""
    n = len(idx)
    lo, hi = off, off + ws
    done = np.zeros(n, bool)
    order = []
    for _ in range(n):
        best, bestw = -1, -1
        for j in range(n):
            if done[j]:
                continue
            ok = True
            for p in range(n):
                if p == j or done[p]:
                    continue
                if idx[p] > idx[j] and lo[p] < hi[j] and lo[j] < hi[p]:
                    ok = False
                    break
            if ok and ws[j] > bestw:
                bestw, best = ws[j], j
        order.append(best)
        done[best] = True
    return np.array(order, int)


def make_plan(centers, radii):
    """Per-core ordered circle lists + global slot width profile."""
    percore = []
    for core in range(N_CORES):
        idx, ws, off = _core_circles(centers, radii, core)
        o = _greedy_f2b(idx, ws, off)
        percore.append((idx[o], ws[o], off[o]))
    S = max(len(p[0]) for p in percore)
    S = ((S + 7) // 8) * 8
    slotw = np.full(S, WMIN, int)
    for idx, ws, off in percore:
        slotw[:len(ws)] = np.maximum(slotw[:len(ws)], ws)
    return percore, slotw


def _hilo(x):
    hi = x.astype(BF)
    lo = (x - hi.astype(np.float64)).astype(BF)
    return hi, lo


def make_inputs(centers, radii, colors, plan):
    percore, slotw = plan
    S = len(slotw)
    npairs = S // 2
    pairw = [int(slotw[2 * i] + slotw[2 * i + 1]) for i in range(npairs)]
    assert all(pw <= 448 for pw in pairw)
    rhs_len = sum(pairw)
    pair_start = np.concatenate([[0], np.cumsum(pairw)]).astype(int)

    cy = centers[:, 1].astype(np.float64)
    cx = centers[:, 0].astype(np.float64)
    r = radii.astype(np.float64)
    col = colors.astype(np.float64)

    ins = []
    for core in range(N_CORES):
        y0 = ROWS * core
        ids, ws, offs_c = percore[core]
        n = len(ids)
        scal = np.zeros((ROWS, S * 4), np.float32)
        offs = np.zeros((1, S), np.int32)
        lhsT = np.zeros((8, npairs * ROWS), BF)
        rhs = np.zeros((8, rhs_len), BF)
        p = y0 + np.arange(ROWS, dtype=np.float64) + 0.5
        for k in range(n):
            i = ids[k]
            vk = int(slotw[k])
            off = int(np.clip(offs_c[k] + (ws[k] - vk) // 2, 0, W - vk))
            offs[0, k] = off
            al = col[i, 3]
            scal[:, k * 4 + 0] = -al
            scal[:, k * 4 + 1] = al * col[i, 0]
            scal[:, k * 4 + 2] = al * col[i, 1]
            scal[:, k * 4 + 3] = al * col[i, 2]
            j = off + np.arange(vk, dtype=np.float64) + 0.5
            a = r[i] / 2.0 - (p - cy[i]) ** 2 / r[i]
            b = r[i] / 2.0 - (j - cx[i]) ** 2 / r[i]
            ah, alo = _hilo(a)
            bh, blo = _hilo(b)
            pair, half = divmod(k, 2)
            rb = 4 * half
            ls = slice(pair * ROWS, (pair + 1) * ROWS)
            lhsT[rb + 0, ls] = ah
            lhsT[rb + 1, ls] = alo
            lhsT[rb + 2, ls] = 1.0
            lhsT[rb + 3, ls] = 1.0
            c0 = pair_start[pair] + (0 if half == 0 else int(slotw[2 * pair]))
            rs = slice(c0, c0 + vk)
            rhs[rb + 0, rs] = 1.0
            rhs[rb + 1, rs] = 1.0
            rhs[rb + 2, rs] = bh
            rhs[rb + 3, rs] = blo
        ins.append({"scal": scal, "offs": offs, "lhsT": lhsT, "rhs": rhs})
    return ins


# ------------------------------------------------------------- device build
def build_nc(slotw):
    slotw = [int(v) for v in slotw]
    S = len(slotw)
    npairs = S // 2
    pairw = [slotw[2 * i] + slotw[2 * i + 1] for i in range(npairs)]
    pair_start = [0]
    for pw in pairw:
        pair_start.append(pair_start[-1] + pw)
    rhs_len = pair_start[-1]
    ngroups = (S + 7) // 8

    nc = bacc.Bacc("TRN2", target_bir_lowering=False, debug=False,
                   num_devices=N_CORES)
    scal_d = nc.dram_tensor("scal", [ROWS, S * 4], F32,
                            kind="ExternalInput").ap()
    offs_d = nc.dram_tensor("offs", [1, S], I32, kind="ExternalInput").ap()
    lhsT_d = nc.dram_tensor("lhsT", [8, npairs * ROWS], BF16,
                            kind="ExternalInput").ap()
    rhs_d = nc.dram_tensor("rhs", [8, rhs_len], BF16,
                           kind="ExternalInput").ap()
    out_d = nc.dram_tensor("out", [ROWS, 4 * W], F16,
                           kind="ExternalOutput").ap()

    with TileContext(nc) as tc:
        T = nc.alloc_sbuf_tensor("T", [ROWS, W], F16).ap()
        CR = nc.alloc_sbuf_tensor("CR", [ROWS, W], F16).ap()
        CG = nc.alloc_sbuf_tensor("CG", [ROWS, W], F16).ap()
        CB = nc.alloc_sbuf_tensor("CB", [ROWS, W], F16).ap()
        AT = nc.alloc_sbuf_tensor("AT", [ROWS, W], F16).ap()
        covr = nc.alloc_sbuf_tensor("covr", [ROWS, 2 * 1792], F16).ap()
        wr = nc.alloc_sbuf_tensor("wr", [ROWS, 8 * WCAP], F16).ap()
        mt = nc.alloc_sbuf_tensor("mt", [ROWS, 16 * WCAP], F16).ap()
        mr = nc.alloc_sbuf_tensor("mr", [ROWS, 4 * 3 * WCAP], F16).ap()
        scal_sb = nc.alloc_sbuf_tensor("scal_sb", [ROWS, S * 4], F32).ap()
        offs_sb = nc.alloc_sbuf_tensor("offs_sb", [1, S], I32).ap()

        nc.sync.dma_start(scal_sb, scal_d)
        nc.sync.dma_start(offs_sb, offs_d)
        nc.vector.memset(T, 1.0)
        nc.vector.memset(CR, 0.0)
        nc.gpsimd.memset(CG, 0.0)
        nc.gpsimd.memset(CB, 0.0)

        CPLANES = (CR, CG, CB)

        with (
            tc.tile_pool(name="psum", bufs=2, space="PSUM") as psum_pool,
            tc.tile_pool(name="ops", bufs=3) as oppool,
        ):
            pend = None  # (slot k, width, pool-offset) awaiting C adds
            for g in range(ngroups):
                k0 = g * 8
                p0 = k0 // 2
                gw = pair_start[p0 + 4] - pair_start[p0]
                lh_t = oppool.tile([8, 4 * ROWS], BF16, tag="lh")
                rh_t = oppool.tile([8, 1792], BF16, tag="rh")
                nc.sync.dma_start(lh_t, lhsT_d[:, p0 * ROWS:(p0 + 4) * ROWS])
                nc.sync.dma_start(rh_t[:, :gw],
                                  rhs_d[:, pair_start[p0]:pair_start[p0 + 4]])
                pt = psum_pool.tile([ROWS, 4 * 512], F32)
                cbase = (g % 2) * 1792
                rpos = 0
                for i in range(4):
                    pw = pairw[p0 + i]
                    nc.tensor.matmul(
                        pt[:, i * 512:i * 512 + pw],
                        lh_t[:, i * ROWS:(i + 1) * ROWS],
                        rh_t[:, rpos:rpos + pw],
                        start=True, stop=True)
                    nc.scalar.activation(
                        covr[:, cbase + rpos:cbase + rpos + pw],
                        pt[:, i * 512:i * 512 + pw], AF.Sigmoid)
                    rpos += pw

                # mtau for all 8 slots of the group (DVE, static rings)
                rpos = 0
                for j in range(8):
                    k = k0 + j
                    vk = slotw[k]
                    mtv = mt[:, (k % 16) * WCAP:(k % 16) * WCAP + vk]
                    nc.vector.tensor_scalar(
                        mtv, covr[:, cbase + rpos:cbase + rpos + vk],
                        scal_sb[:, k * 4:k * 4 + 1], 1.0, OP.mult, OP.add)
                    rpos += vk

                # offsets for the group on Pool
                gregs = [nc.gpsimd.alloc_register(f"off_{k0}_{i}")
                         for i in range(8)]
                nc.gpsimd.reg_load(gregs, offs_sb[0:1, k0:k0 + 8])
                goff = [nc.gpsimd.snap(gregs[j], donate=True, min_val=0,
                                       max_val=W - slotw[k0 + j])
                        for j in range(8)]

                rpos = 0
                for j in range(8):
                    k = k0 + j
                    vk = slotw[k]
                    cov = covr[:, cbase + rpos:cbase + rpos + vk]
                    rpos += vk
                    wv = wr[:, (k % 8) * WCAP:(k % 8) * WCAP + vk]
                    mtv = mt[:, (k % 16) * WCAP:(k % 16) * WCAP + vk]
                    tw = T[:, bass.ds(goff[j], vk)]
                    # Pool: w = T*cov ; T *= mtau ; then pend C adds
                    nc.gpsimd.tensor_tensor(wv, tw, cov, OP.mult)
                    nc.gpsimd.tensor_tensor(tw, tw, mtv, OP.mult)
                    if pend is not None:
                        kp, vp, offp = pend
                        mb = (kp % 4) * 3 * WCAP
                        for ch in range(3):
                            cw = CPLANES[ch][:, bass.ds(offp, vp)]
                            nc.gpsimd.tensor_tensor(
                                cw, cw,
                                mr[:, mb + ch * WCAP:mb + ch * WCAP + vp],
                                OP.add)
                    # DVE premultiplies for this slot
                    mb = (k % 4) * 3 * WCAP
                    for ch in range(3):
                        nc.vector.tensor_scalar(
                            mr[:, mb + ch * WCAP:mb + ch * WCAP + vk],
                            wv, scal_sb[:, k * 4 + 1 + ch:k * 4 + 2 + ch],
                            0.0, OP.mult, OP.add)
                    pend = (k, vk, goff[j])

            if pend is not None:
                kp, vp, offp = pend
                mb = (kp % 4) * 3 * WCAP
                for ch in range(3):
                    cw = CPLANES[ch][:, bass.ds(offp, vp)]
                    nc.gpsimd.tensor_tensor(
                        cw, cw, mr[:, mb + ch * WCAP:mb + ch * WCAP + vp],
                        OP.add)
                pend = None

        # A = 1 - T, then 4 plane DMAs
        nc.vector.tensor_scalar(AT, T, -1.0, 1.0, OP.mult, OP.add)
        for q, plane in enumerate((CR, CG, CB, AT)):
            nc.sync.dma_start(out_d[:, q * W:(q + 1) * W], plane)

    nc.compile()
    return nc


_CACHE = {}


def _get_nc(slotw):
    key = tuple(int(v) for v in slotw)
    if key not in _CACHE:
        _CACHE[key] = build_nc(slotw)
    return _CACHE[key]


def kernel(centers, radii, colors):
    centers = np.asarray(centers, np.float32)
    radii = np.asarray(radii, np.float32)
    colors = np.asarray(colors, np.float32)

    plan = make_plan(centers, radii)
    nc = _get_nc(plan[1])
    ins = make_inputs(centers, radii, colors, plan)
    res = bass_utils.run_bass_kernel_spmd(nc, ins, list(range(N_CORES)),
                                          trace=False)
    out = np.empty((H, W, 4), np.float32)
    for c in range(N_CORES):
        planes = res.results[c]["out"].astype(np.float32)  # [128, 4*W]
        for ch in range(4):
            out[c * ROWS:(c + 1) * ROWS, :, ch] = planes[:, ch * W:(ch + 1) * W]
    return out


# revision 11
# speedup vs baseline: 1.2948x; 1.2592x over previous
"""DiffVG-style circle renderer on 8 Trainium2 NeuronCores.

Strategy: shard the 1024x1024 image by rows (128 rows per core). Each core
composites the circles whose vertical span intersects its row band,
front-to-back with transmittance T:

    cov = sigmoid(r - d^2/r)          ~= sigmoid(2(r - d)) near the edge
    w   = T * cov                      (w ring, fp16)
    T  *= (1 - a*cov)                  (mtau ring premultiplied on DVE)
    C_ch += (a*col_ch) * w             (premultiplied m_ch on DVE)

Front-to-back order is relaxed: circles whose column windows don't overlap
commute, so each core emits a width-descending order compatible with the
z partial order. Slot k's window width is the max over cores of the k-th
emitted circle width (compile-time constant); offsets are runtime data.

Engine split (all dynamic-window ops as cheap Pool tensor_tensor):
  PE     z = (r^2 - d^2)/r outer-sum; two circles per K=8 matmul
         (bf16 hi/lo split operands), bias folded in -> no sqrt pass
  ACT    per-pair sigmoid PSUM -> fp16 cov ring
  DVE    static-ring tensor_scalar premultiplies: mtau = 1 - a*cov,
         m_ch = (a col_ch) * w
  Pool   w = T*cov, T *= mtau, C_ch += m_ch  on dynamic windows
State T, CR, CG, CB are fp16 planes; output = 4 fp16 planes DMA'd out,
assembled/converted to f32 on host.
"""

import sys

if "/opt/trn_rl_repo" not in sys.path:
    sys.path.insert(0, "/opt/trn_rl_repo")

import numpy as np
import ml_dtypes

import concourse.bass as bass
import concourse.bacc as bacc
import concourse.mybir as mybir
from concourse.tile import TileContext
from concourse import bass_utils

H = 1024
W = 1024
ROWS = 128
N_CORES = 8
MARGIN = 6.0
ROUND = 16
WMIN = 32
WCAP = 224
F32 = mybir.dt.float32
F16 = mybir.dt.float16
BF16 = mybir.dt.bfloat16
I32 = mybir.dt.int32
AF = mybir.ActivationFunctionType
OP = mybir.AluOpType
BF = ml_dtypes.bfloat16


# ---------------------------------------------------------------- host plan
def _core_circles(centers, radii, core):
    """Kept circle indices + cap-clipped rounded widths + offsets."""
    y0 = ROWS * core
    cy = centers[:, 1].astype(np.float64)
    cx = centers[:, 0].astype(np.float64)
    r = radii.astype(np.float64)
    keep = (cy + r + MARGIN >= y0 + 0.5) & (cy - r - MARGIN <= y0 + ROWS - 0.5)
    idx = np.where(keep)[0]
    dymin = np.maximum(0.0, np.maximum(y0 + 0.5 - cy[idx],
                                       cy[idx] - (y0 + ROWS - 0.5)))
    rm = r[idx] + MARGIN
    halfw = np.sqrt(np.maximum(rm * rm - dymin * dymin, 4.0))
    ws = np.clip(np.ceil(2.0 * halfw / ROUND) * ROUND, WMIN, WCAP).astype(int)
    off = np.clip(np.round(cx[idx] - ws / 2.0), 0, W - ws).astype(int)
    return idx, ws, off


def _greedy_f2b(idx, ws, off):
    """Front-to-back (topmost first) order, widest-available-first among
    circles whose later-drawn column-overlapping circles are all emitted."""
    n = len(idx)
    lo, hi = off, off + ws
    done = np.zeros(n, bool)
    order = []
    for _ in range(n):
        best, bestw = -1, -1
        for j in range(n):
            if done[j]:
                continue
            ok = True
            for p in range(n):
                if p == j or done[p]:
                    continue
                if idx[p] > idx[j] and lo[p] < hi[j] and lo[j] < hi[p]:
                    ok = False
                    break
            if ok and ws[j] > bestw:
                bestw, best = ws[j], j
        order.append(best)
        done[best] = True
    return np.array(order, int)


def make_plan(centers, radii):
    """Per-core ordered circle lists + global slot width profile."""
    percore = []
    for core in range(N_CORES):
        idx, ws, off = _core_circles(centers, radii, core)
        o = _greedy_f2b(idx, ws, off)
        percore.append((idx[o], ws[o], off[o]))
    S = max(len(p[0]) for p in percore)
    S = ((S + 7) // 8) * 8
    slotw = np.full(S, WMIN, int)
    for idx, ws, off in percore:
        slotw[:len(ws)] = np.maximum(slotw[:len(ws)], ws)
    return percore, slotw


def _hilo(x):
    hi = x.astype(BF)
    lo = (x - hi.astype(np.float64)).astype(BF)
    return hi, lo


def make_inputs(centers, radii, colors, plan):
    percore, slotw = plan
    S = len(slotw)
    npairs = S // 2
    pairw = [int(slotw[2 * i] + slotw[2 * i + 1]) for i in range(npairs)]
    assert all(pw <= 448 for pw in pairw)
    rhs_len = sum(pairw)
    pair_start = np.concatenate([[0], np.cumsum(pairw)]).astype(int)

    cy = centers[:, 1].astype(np.float64)
    cx = centers[:, 0].astype(np.float64)
    r = radii.astype(np.float64)
    col = colors.astype(np.float64)

    ins = []
    for core in range(N_CORES):
        y0 = ROWS * core
        ids, ws, offs_c = percore[core]
        n = len(ids)
        scal = np.zeros((ROWS, S * 4), np.float32)
        offs = np.zeros((1, S), np.int32)
        lhsT = np.zeros((8, npairs * ROWS), BF)
        rhs = np.zeros((8, rhs_len), BF)
        p = y0 + np.arange(ROWS, dtype=np.float64) + 0.5
        for k in range(n):
            i = ids[k]
            vk = int(slotw[k])
            off = int(np.clip(offs_c[k] + (ws[k] - vk) // 2, 0, W - vk))
            offs[0, k] = off
            al = col[i, 3]
            scal[:, k * 4 + 0] = -al
            scal[:, k * 4 + 1] = al * col[i, 0]
            scal[:, k * 4 + 2] = al * col[i, 1]
            scal[:, k * 4 + 3] = al * col[i, 2]
            j = off + np.arange(vk, dtype=np.float64) + 0.5
            a = r[i] / 2.0 - (p - cy[i]) ** 2 / r[i]
            b = r[i] / 2.0 - (j - cx[i]) ** 2 / r[i]
            ah, alo = _hilo(a)
            bh, blo = _hilo(b)
            pair, half = divmod(k, 2)
            rb = 4 * half
            ls = slice(pair * ROWS, (pair + 1) * ROWS)
            lhsT[rb + 0, ls] = ah
            lhsT[rb + 1, ls] = alo
            lhsT[rb + 2, ls] = 1.0
            lhsT[rb + 3, ls] = 1.0
            c0 = pair_start[pair] + (0 if half == 0 else int(slotw[2 * pair]))
            rs = slice(c0, c0 + vk)
            rhs[rb + 0, rs] = 1.0
            rhs[rb + 1, rs] = 1.0
            rhs[rb + 2, rs] = bh
            rhs[rb + 3, rs] = blo
        ins.append({"scal": scal, "sc16": scal.astype(np.float16),
                    "offs": offs, "lhsT": lhsT, "rhs": rhs})
    return ins


# ------------------------------------------------------------- device build
def build_nc(slotw):
    slotw = [int(v) for v in slotw]
    S = len(slotw)
    npairs = S // 2
    pairw = [slotw[2 * i] + slotw[2 * i + 1] for i in range(npairs)]
    pair_start = [0]
    for pw in pairw:
        pair_start.append(pair_start[-1] + pw)
    rhs_len = pair_start[-1]
    ngroups = (S + 7) // 8

    nc = bacc.Bacc("TRN2", target_bir_lowering=False, debug=False,
                   num_devices=N_CORES)
    scal_d = nc.dram_tensor("scal", [ROWS, S * 4], F32,
                            kind="ExternalInput").ap()
    sc16_d = nc.dram_tensor("sc16", [ROWS, S * 4], F16,
                            kind="ExternalInput").ap()
    offs_d = nc.dram_tensor("offs", [1, S], I32, kind="ExternalInput").ap()
    lhsT_d = nc.dram_tensor("lhsT", [8, npairs * ROWS], BF16,
                            kind="ExternalInput").ap()
    rhs_d = nc.dram_tensor("rhs", [8, rhs_len], BF16,
                           kind="ExternalInput").ap()
    out_d = nc.dram_tensor("out", [ROWS, 4 * W], F16,
                           kind="ExternalOutput").ap()

    with TileContext(nc) as tc:
        T = nc.alloc_sbuf_tensor("T", [ROWS, W], F16).ap()
        CC = nc.alloc_sbuf_tensor("CC", [ROWS, 3 * W], F16).ap()
        AT = nc.alloc_sbuf_tensor("AT", [ROWS, W], F16).ap()
        covr = nc.alloc_sbuf_tensor("covr", [ROWS, 2 * 1792], F16).ap()
        wr = nc.alloc_sbuf_tensor("wr", [ROWS, 8 * WCAP], F16).ap()
        mt = nc.alloc_sbuf_tensor("mt", [ROWS, 16 * WCAP], F16).ap()
        mr = nc.alloc_sbuf_tensor("mr", [ROWS, 4 * 3 * WCAP], F16).ap()
        scal_sb = nc.alloc_sbuf_tensor("scal_sb", [ROWS, S * 4], F32).ap()
        sc16_sb = nc.alloc_sbuf_tensor("sc16_sb", [ROWS, S * 4], F16).ap()
        offs_sb = nc.alloc_sbuf_tensor("offs_sb", [1, S], I32).ap()

        nc.sync.dma_start(scal_sb, scal_d)
        nc.sync.dma_start(sc16_sb, sc16_d)
        nc.sync.dma_start(offs_sb, offs_d)
        nc.vector.memset(T, 1.0)
        nc.vector.memset(CC, 0.0)

        CC3 = CC.rearrange("p (c x) -> p c x", x=W)
        mr3 = mr.rearrange("p (s x) -> p s x", x=WCAP)

        with (
            tc.tile_pool(name="psum", bufs=2, space="PSUM") as psum_pool,
            tc.tile_pool(name="ops", bufs=3) as oppool,
        ):
            pend = None  # (slot k, width, pool-offset) awaiting C adds
            for g in range(ngroups):
                k0 = g * 8
                p0 = k0 // 2
                gw = pair_start[p0 + 4] - pair_start[p0]
                lh_t = oppool.tile([8, 4 * ROWS], BF16, tag="lh")
                rh_t = oppool.tile([8, 1792], BF16, tag="rh")
                nc.sync.dma_start(lh_t, lhsT_d[:, p0 * ROWS:(p0 + 4) * ROWS])
                nc.sync.dma_start(rh_t[:, :gw],
                                  rhs_d[:, pair_start[p0]:pair_start[p0 + 4]])
                pt = psum_pool.tile([ROWS, 4 * 512], F32)
                cbase = (g % 2) * 1792
                rpos = 0
                for i in range(4):
                    pw = pairw[p0 + i]
                    nc.tensor.matmul(
                        pt[:, i * 512:i * 512 + pw],
                        lh_t[:, i * ROWS:(i + 1) * ROWS],
                        rh_t[:, rpos:rpos + pw],
                        start=True, stop=True)
                    nc.scalar.activation(
                        covr[:, cbase + rpos:cbase + rpos + pw],
                        pt[:, i * 512:i * 512 + pw], AF.Sigmoid)
                    rpos += pw

                # mtau = 1 - a*cov for all 8 slots of the group (ACT copy)
                rpos = 0
                for j in range(8):
                    k = k0 + j
                    vk = slotw[k]
                    mtv = mt[:, (k % 16) * WCAP:(k % 16) * WCAP + vk]
                    nc.scalar.activation(
                        mtv, covr[:, cbase + rpos:cbase + rpos + vk],
                        AF.Copy, bias=1.0,
                        scale=scal_sb[:, k * 4:k * 4 + 1])
                    rpos += vk

                # offsets for the group on Pool (chain) and DVE (C adds)
                gregs = [nc.gpsimd.alloc_register(f"off_{k0}_{i}")
                         for i in range(8)]
                nc.gpsimd.reg_load(gregs, offs_sb[0:1, k0:k0 + 8])
                goff = [nc.gpsimd.snap(gregs[j], donate=True, min_val=0,
                                       max_val=W - slotw[k0 + j])
                        for j in range(8)]
                vregs = [nc.vector.alloc_register(f"voff_{k0}_{i}")
                         for i in range(8)]
                nc.vector.reg_load(vregs, offs_sb[0:1, k0:k0 + 8])
                voff = [nc.vector.snap(vregs[j], donate=True, min_val=0,
                                       max_val=W - slotw[k0 + j])
                        for j in range(8)]

                rpos = 0
                for j in range(8):
                    k = k0 + j
                    vk = slotw[k]
                    cov = covr[:, cbase + rpos:cbase + rpos + vk]
                    rpos += vk
                    wv = wr[:, (k % 8) * WCAP:(k % 8) * WCAP + vk]
                    mtv = mt[:, (k % 16) * WCAP:(k % 16) * WCAP + vk]
                    tw = T[:, bass.ds(goff[j], vk)]
                    # Pool: w = T*cov ; T *= mtau  (serial chain stays on Pool)
                    nc.gpsimd.tensor_tensor(wv, tw, cov, OP.mult)
                    nc.gpsimd.tensor_tensor(tw, tw, mtv, OP.mult)
                    if pend is not None:
                        kp, vp, offp = pend
                        qv = CC3[:, :, bass.ds(offp, vp)]
                        mp = mr3[:, (kp % 4) * 3:(kp % 4) * 3 + 3, :vp]
                        nc.vector.tensor_tensor(qv, qv, mp, OP.add)
                    # DVE broadcast premultiply: m3[ch] = (a col_ch) * w
                    w3 = wv.rearrange("p (c x) -> p c x", c=1)
                    sv3 = sc16_sb[:, k * 4 + 1:k * 4 + 4].rearrange(
                        "p (c x) -> p c x", x=1)
                    b0, b1 = bass.broadcast_tensor_aps(w3, sv3)
                    nc.vector.tensor_tensor(
                        mr3[:, (k % 4) * 3:(k % 4) * 3 + 3, :vk],
                        b0, b1, OP.mult)
                    pend = (k, vk, voff[j])

            if pend is not None:
                kp, vp, offp = pend
                qv = CC3[:, :, bass.ds(offp, vp)]
                mp = mr3[:, (kp % 4) * 3:(kp % 4) * 3 + 3, :vp]
                nc.vector.tensor_tensor(qv, qv, mp, OP.add)
                pend = None

        # A = 1 - T, then plane DMAs
        nc.vector.tensor_scalar(AT, T, -1.0, 1.0, OP.mult, OP.add)
        nc.sync.dma_start(out_d[:, 0:3 * W], CC)
        nc.sync.dma_start(out_d[:, 3 * W:4 * W], AT)

    nc.compile()
    return nc


_CACHE = {}


def _get_nc(slotw):
    key = tuple(int(v) for v in slotw)
    if key not in _CACHE:
        _CACHE[key] = build_nc(slotw)
    return _CACHE[key]


def kernel(centers, radii, colors):
    centers = np.asarray(centers, np.float32)
    radii = np.asarray(radii, np.float32)
    colors = np.asarray(colors, np.float32)

    plan = make_plan(centers, radii)
    nc = _get_nc(plan[1])
    ins = make_inputs(centers, radii, colors, plan)
    res = bass_utils.run_bass_kernel_spmd(nc, ins, list(range(N_CORES)),
                                          trace=False)
    out = np.empty((H, W, 4), np.float32)
    for c in range(N_CORES):
        planes = res.results[c]["out"].astype(np.float32)  # [128, 4*W]
        for ch in range(4):
            out[c * ROWS:(c + 1) * ROWS, :, ch] = planes[:, ch * W:(ch + 1) * W]
    return out


# revision 14
# speedup vs baseline: 1.5768x; 1.2178x over previous
"""DiffVG-style circle renderer on 8 Trainium2 NeuronCores.

Strategy: shard the 1024x1024 image by rows (128 rows per core). Each core
composites the circles whose vertical span intersects its row band,
front-to-back with transmittance T:

    cov = sigmoid(r - d^2/r)          ~= sigmoid(2(r - d)) near the edge
    w   = T * cov                      (w ring, fp16)
    T  *= (1 - a*cov)                  (mtau ring premultiplied on DVE)
    C_ch += (a*col_ch) * w             (premultiplied m_ch on DVE)

Front-to-back order is relaxed: circles whose column windows don't overlap
commute, so each core emits a width-descending order compatible with the
z partial order. Slot k's window width is the max over cores of the k-th
emitted circle width (compile-time constant); offsets are runtime data.

Engine split (all dynamic-window ops as cheap Pool tensor_tensor):
  PE     z = (r^2 - d^2)/r outer-sum; two circles per K=8 matmul
         (bf16 hi/lo split operands), bias folded in -> no sqrt pass
  ACT    per-pair sigmoid PSUM -> fp16 cov ring
  DVE    static-ring tensor_scalar premultiplies: mtau = 1 - a*cov,
         m_ch = (a col_ch) * w
  Pool   w = T*cov, T *= mtau, C_ch += m_ch  on dynamic windows
State T, CR, CG, CB are fp16 planes; output = 4 fp16 planes DMA'd out,
assembled/converted to f32 on host.
"""

import sys

if "/opt/trn_rl_repo" not in sys.path:
    sys.path.insert(0, "/opt/trn_rl_repo")

import numpy as np
import ml_dtypes

import concourse.bass as bass
import concourse.bacc as bacc
import concourse.mybir as mybir
from concourse.tile import TileContext
from concourse import bass_utils

H = 1024
W = 1024
ROWS = 128
N_CORES = 8
MARGIN = 6.0
ROUND = 16
WMIN = 32
WCAP = 224
F32 = mybir.dt.float32
F16 = mybir.dt.float16
BF16 = mybir.dt.bfloat16
I32 = mybir.dt.int32
AF = mybir.ActivationFunctionType
OP = mybir.AluOpType
BF = ml_dtypes.bfloat16


# ---------------------------------------------------------------- host plan
def _core_circles(centers, radii, core):
    """Kept circle indices + cap-clipped rounded widths + offsets."""
    y0 = ROWS * core
    cy = centers[:, 1].astype(np.float64)
    cx = centers[:, 0].astype(np.float64)
    r = radii.astype(np.float64)
    keep = (cy + r + MARGIN >= y0 + 0.5) & (cy - r - MARGIN <= y0 + ROWS - 0.5)
    idx = np.where(keep)[0]
    dymin = np.maximum(0.0, np.maximum(y0 + 0.5 - cy[idx],
                                       cy[idx] - (y0 + ROWS - 0.5)))
    rm = r[idx] + MARGIN
    halfw = np.sqrt(np.maximum(rm * rm - dymin * dymin, 4.0))
    ws = np.clip(np.ceil(2.0 * halfw / ROUND) * ROUND, WMIN, WCAP).astype(int)
    off = np.clip(np.round(cx[idx] - ws / 2.0), 0, W - ws).astype(int)
    return idx, ws, off


def _greedy_f2b(idx, ws, off):
    """Front-to-back (topmost first) order, widest-available-first among
    circles whose later-drawn column-overlapping circles are all emitted."""
    n = len(idx)
    lo, hi = off, off + ws
    done = np.zeros(n, bool)
    order = []
    for _ in range(n):
        best, bestw = -1, -1
        for j in range(n):
            if done[j]:
                continue
            ok = True
            for p in range(n):
                if p == j or done[p]:
                    continue
                if idx[p] > idx[j] and lo[p] < hi[j] and lo[j] < hi[p]:
                    ok = False
                    break
            if ok and ws[j] > bestw:
                bestw, best = ws[j], j
        order.append(best)
        done[best] = True
    return np.array(order, int)


def make_plan(centers, radii):
    """Per-core ordered circle lists + global slot width profile."""
    percore = []
    for core in range(N_CORES):
        idx, ws, off = _core_circles(centers, radii, core)
        o = _greedy_f2b(idx, ws, off)
        percore.append((idx[o], ws[o], off[o]))
    S = max(len(p[0]) for p in percore)
    S = ((S + 7) // 8) * 8
    slotw = np.full(S, WMIN, int)
    for idx, ws, off in percore:
        slotw[:len(ws)] = np.maximum(slotw[:len(ws)], ws)
    return percore, slotw


def _hilo(x):
    hi = x.astype(BF)
    lo = (x - hi.astype(np.float64)).astype(BF)
    return hi, lo


def make_inputs(centers, radii, colors, plan):
    percore, slotw = plan
    S = len(slotw)
    npairs = S // 2
    pairw = [int(slotw[2 * i] + slotw[2 * i + 1]) for i in range(npairs)]
    assert all(pw <= 448 for pw in pairw)
    rhs_len = sum(pairw)
    pair_start = np.concatenate([[0], np.cumsum(pairw)]).astype(int)

    cy = centers[:, 1].astype(np.float64)
    cx = centers[:, 0].astype(np.float64)
    r = radii.astype(np.float64)
    col = colors.astype(np.float64)

    ins = []
    for core in range(N_CORES):
        y0 = ROWS * core
        ids, ws, offs_c = percore[core]
        n = len(ids)
        scal = np.zeros((ROWS, S * 4), np.float32)
        offs = np.zeros((1, S), np.int32)
        lhsT = np.zeros((8, npairs * ROWS), BF)
        rhs = np.zeros((8, rhs_len), BF)
        p = y0 + np.arange(ROWS, dtype=np.float64) + 0.5
        for k in range(n):
            i = ids[k]
            vk = int(slotw[k])
            off = int(np.clip(offs_c[k] + (ws[k] - vk) // 2, 0, W - vk))
            offs[0, k] = off
            al = col[i, 3]
            scal[:, k * 4 + 0] = -al
            scal[:, k * 4 + 1] = al * col[i, 0]
            scal[:, k * 4 + 2] = al * col[i, 1]
            scal[:, k * 4 + 3] = al * col[i, 2]
            j = off + np.arange(vk, dtype=np.float64) + 0.5
            a = r[i] / 2.0 - (p - cy[i]) ** 2 / r[i]
            b = r[i] / 2.0 - (j - cx[i]) ** 2 / r[i]
            ah, alo = _hilo(a)
            bh, blo = _hilo(b)
            pair, half = divmod(k, 2)
            rb = 4 * half
            ls = slice(pair * ROWS, (pair + 1) * ROWS)
            lhsT[rb + 0, ls] = ah
            lhsT[rb + 1, ls] = alo
            lhsT[rb + 2, ls] = 1.0
            lhsT[rb + 3, ls] = 1.0
            c0 = pair_start[pair] + (0 if half == 0 else int(slotw[2 * pair]))
            rs = slice(c0, c0 + vk)
            rhs[rb + 0, rs] = 1.0
            rhs[rb + 1, rs] = 1.0
            rhs[rb + 2, rs] = bh
            rhs[rb + 3, rs] = blo
        ins.append({"scal": scal, "sc16": scal.astype(np.float16),
                    "offs": offs, "lhsT": lhsT, "rhs": rhs})
    return ins


# ------------------------------------------------------------- device build
def build_nc(slotw):
    slotw = [int(v) for v in slotw]
    S = len(slotw)
    npairs = S // 2
    pairw = [slotw[2 * i] + slotw[2 * i + 1] for i in range(npairs)]
    pair_start = [0]
    for pw in pairw:
        pair_start.append(pair_start[-1] + pw)
    rhs_len = pair_start[-1]
    ngroups = (S + 7) // 8

    nc = bacc.Bacc("TRN2", target_bir_lowering=False, debug=False,
                   num_devices=N_CORES)
    scal_d = nc.dram_tensor("scal", [ROWS, S * 4], F32,
                            kind="ExternalInput").ap()
    sc16_d = nc.dram_tensor("sc16", [ROWS, S * 4], F16,
                            kind="ExternalInput").ap()
    offs_d = nc.dram_tensor("offs", [1, S], I32, kind="ExternalInput").ap()
    lhsT_d = nc.dram_tensor("lhsT", [8, npairs * ROWS], BF16,
                            kind="ExternalInput").ap()
    rhs_d = nc.dram_tensor("rhs", [8, rhs_len], BF16,
                           kind="ExternalInput").ap()
    out_d = nc.dram_tensor("out", [ROWS, 4 * W], F16,
                           kind="ExternalOutput").ap()

    with TileContext(nc) as tc:
        T = nc.alloc_sbuf_tensor("T", [ROWS, W], F16).ap()
        CC = nc.alloc_sbuf_tensor("CC", [ROWS, 3 * W], F16).ap()
        AT = nc.alloc_sbuf_tensor("AT", [ROWS, W], F16).ap()
        covr = nc.alloc_sbuf_tensor("covr", [ROWS, 2 * 1792], F16).ap()
        wr = nc.alloc_sbuf_tensor("wr", [ROWS, 8 * WCAP], F16).ap()
        mr = nc.alloc_sbuf_tensor("mr", [ROWS, 4 * 3 * WCAP], F16).ap()
        scal_sb = nc.alloc_sbuf_tensor("scal_sb", [ROWS, S * 4], F32).ap()
        sc16_sb = nc.alloc_sbuf_tensor("sc16_sb", [ROWS, S * 4], F16).ap()
        offs_sb = nc.alloc_sbuf_tensor("offs_sb", [1, S], I32).ap()

        nc.sync.dma_start(scal_sb, scal_d)
        nc.sync.dma_start(sc16_sb, sc16_d)
        nc.sync.dma_start(offs_sb, offs_d)
        nc.vector.memset(T, 1.0)
        nc.vector.memset(CC, 0.0)

        CC3 = CC.rearrange("p (c x) -> p c x", x=W)
        mr3 = mr.rearrange("p (s x) -> p s x", x=WCAP)

        with (
            tc.tile_pool(name="psum", bufs=2, space="PSUM") as psum_pool,
            tc.tile_pool(name="ops", bufs=3) as oppool,
        ):
            pend = None  # (slot k, width, pool-offset) awaiting C adds
            for g in range(ngroups):
                k0 = g * 8
                p0 = k0 // 2
                gw = pair_start[p0 + 4] - pair_start[p0]
                lh_t = oppool.tile([8, 4 * ROWS], BF16, tag="lh")
                rh_t = oppool.tile([8, 1792], BF16, tag="rh")
                nc.sync.dma_start(lh_t, lhsT_d[:, p0 * ROWS:(p0 + 4) * ROWS])
                nc.sync.dma_start(rh_t[:, :gw],
                                  rhs_d[:, pair_start[p0]:pair_start[p0 + 4]])
                pt = psum_pool.tile([ROWS, 4 * 512], F32)
                cbase = (g % 2) * 1792
                rpos = 0
                for i in range(4):
                    pw = pairw[p0 + i]
                    nc.tensor.matmul(
                        pt[:, i * 512:i * 512 + pw],
                        lh_t[:, i * ROWS:(i + 1) * ROWS],
                        rh_t[:, rpos:rpos + pw],
                        start=True, stop=True)
                    nc.scalar.activation(
                        covr[:, cbase + rpos:cbase + rpos + pw],
                        pt[:, i * 512:i * 512 + pw], AF.Sigmoid)
                    rpos += pw

                # offsets for the group on Pool (w) and DVE (T chain, C adds)
                gregs = [nc.gpsimd.alloc_register(f"off_{k0}_{i}")
                         for i in range(8)]
                nc.gpsimd.reg_load(gregs, offs_sb[0:1, k0:k0 + 8])
                goff = [nc.gpsimd.snap(gregs[j], donate=True, min_val=0,
                                       max_val=W - slotw[k0 + j])
                        for j in range(8)]
                vregs = [nc.vector.alloc_register(f"voff_{k0}_{i}")
                         for i in range(8)]
                nc.vector.reg_load(vregs, offs_sb[0:1, k0:k0 + 8])
                voff = [nc.vector.snap(vregs[j], donate=True, min_val=0,
                                       max_val=W - slotw[k0 + j])
                        for j in range(8)]

                rpos = 0
                for j in range(8):
                    k = k0 + j
                    vk = slotw[k]
                    cov = covr[:, cbase + rpos:cbase + rpos + vk]
                    rpos += vk
                    wv = wr[:, (k % 8) * WCAP:(k % 8) * WCAP + vk]
                    tw = T[:, bass.ds(goff[j], vk)]
                    # Pool: w = T*cov  (Pool's only slot op)
                    nc.gpsimd.tensor_tensor(wv, tw, cov, OP.mult)
                    # DVE: T = (-a)*w + T  (chain-critical, emit first)
                    twv = T[:, bass.ds(voff[j], vk)]
                    nc.vector.scalar_tensor_tensor(
                        twv, wv, scal_sb[:, k * 4:k * 4 + 1], twv,
                        OP.mult, OP.add)
                    if pend is not None:
                        kp, vp, offp = pend
                        qv = CC3[:, :, bass.ds(offp, vp)]
                        mp = mr3[:, (kp % 4) * 3:(kp % 4) * 3 + 3, :vp]
                        nc.vector.tensor_tensor(qv, qv, mp, OP.add)
                    # ACT: m_B = (a colB) * w
                    mbB = ((k % 4) * 3 + 2) * WCAP
                    nc.scalar.activation(
                        mr[:, mbB:mbB + vk], wv, AF.Copy,
                        scale=scal_sb[:, k * 4 + 3:k * 4 + 4])
                    # DVE broadcast premultiply: m2[ch] = (a col_ch) * w
                    w3 = wv.rearrange("p (c x) -> p c x", c=1)
                    sv2 = sc16_sb[:, k * 4 + 1:k * 4 + 3].rearrange(
                        "p (c x) -> p c x", x=1)
                    b0, b1 = bass.broadcast_tensor_aps(w3, sv2)
                    nc.vector.tensor_tensor(
                        mr3[:, (k % 4) * 3:(k % 4) * 3 + 2, :vk],
                        b0, b1, OP.mult)
                    pend = (k, vk, voff[j])

            if pend is not None:
                kp, vp, offp = pend
                qv = CC3[:, :, bass.ds(offp, vp)]
                mp = mr3[:, (kp % 4) * 3:(kp % 4) * 3 + 3, :vp]
                nc.vector.tensor_tensor(qv, qv, mp, OP.add)
                pend = None

        # A = 1 - T, then plane DMAs
        nc.vector.tensor_scalar(AT, T, -1.0, 1.0, OP.mult, OP.add)
        nc.sync.dma_start(out_d[:, 0:3 * W], CC)
        nc.sync.dma_start(out_d[:, 3 * W:4 * W], AT)

    nc.compile()
    return nc


_CACHE = {}


def _get_nc(slotw):
    key = tuple(int(v) for v in slotw)
    if key not in _CACHE:
        _CACHE[key] = build_nc(slotw)
    return _CACHE[key]


def kernel(centers, radii, colors):
    centers = np.asarray(centers, np.float32)
    radii = np.asarray(radii, np.float32)
    colors = np.asarray(colors, np.float32)

    plan = make_plan(centers, radii)
    nc = _get_nc(plan[1])
    ins = make_inputs(centers, radii, colors, plan)
    res = bass_utils.run_bass_kernel_spmd(nc, ins, list(range(N_CORES)),
                                          trace=False)
    out = np.empty((H, W, 4), np.float32)
    for c in range(N_CORES):
        planes = res.results[c]["out"].astype(np.float32)  # [128, 4*W]
        for ch in range(4):
            out[c * ROWS:(c + 1) * ROWS, :, ch] = planes[:, ch * W:(ch + 1) * W]
    return out


# revision 15
# speedup vs baseline: 1.6040x; 1.0172x over previous
"""DiffVG-style circle renderer on 8 Trainium2 NeuronCores.

Strategy: shard the 1024x1024 image by rows (128 rows per core). Each core
composites the circles whose vertical span intersects its row band,
front-to-back with transmittance T:

    cov = sigmoid(r - d^2/r)          ~= sigmoid(2(r - d)) near the edge
    w   = T * cov                      (w ring, fp16)
    T  *= (1 - a*cov)                  (mtau ring premultiplied on DVE)
    C_ch += (a*col_ch) * w             (premultiplied m_ch on DVE)

Front-to-back order is relaxed: circles whose column windows don't overlap
commute, so each core emits a width-descending order compatible with the
z partial order. Slot k's window width is the max over cores of the k-th
emitted circle width (compile-time constant); offsets are runtime data.

Engine split (all dynamic-window ops as cheap Pool tensor_tensor):
  PE     z = (r^2 - d^2)/r outer-sum; two circles per K=8 matmul
         (bf16 hi/lo split operands), bias folded in -> no sqrt pass
  ACT    per-pair sigmoid PSUM -> fp16 cov ring
  DVE    static-ring tensor_scalar premultiplies: mtau = 1 - a*cov,
         m_ch = (a col_ch) * w
  Pool   w = T*cov, T *= mtau, C_ch += m_ch  on dynamic windows
State T, CR, CG, CB are fp16 planes; output = 4 fp16 planes DMA'd out,
assembled/converted to f32 on host.
"""

import sys

if "/opt/trn_rl_repo" not in sys.path:
    sys.path.insert(0, "/opt/trn_rl_repo")

import numpy as np
import ml_dtypes

import concourse.bass as bass
import concourse.bacc as bacc
import concourse.mybir as mybir
from concourse.tile import TileContext
from concourse import bass_utils

H = 1024
W = 1024
ROWS = 128
N_CORES = 8
MARGIN = 5.0
ROUND = 8
WMIN = 24
WCAP = 224
F32 = mybir.dt.float32
F16 = mybir.dt.float16
BF16 = mybir.dt.bfloat16
I32 = mybir.dt.int32
AF = mybir.ActivationFunctionType
OP = mybir.AluOpType
BF = ml_dtypes.bfloat16


# ---------------------------------------------------------------- host plan
def _core_circles(centers, radii, core):
    """Kept circle indices + cap-clipped rounded widths + offsets."""
    y0 = ROWS * core
    cy = centers[:, 1].astype(np.float64)
    cx = centers[:, 0].astype(np.float64)
    r = radii.astype(np.float64)
    keep = (cy + r + MARGIN >= y0 + 0.5) & (cy - r - MARGIN <= y0 + ROWS - 0.5)
    idx = np.where(keep)[0]
    dymin = np.maximum(0.0, np.maximum(y0 + 0.5 - cy[idx],
                                       cy[idx] - (y0 + ROWS - 0.5)))
    rm = r[idx] + MARGIN
    halfw = np.sqrt(np.maximum(rm * rm - dymin * dymin, 4.0))
    ws = np.clip(np.ceil(2.0 * halfw / ROUND) * ROUND, WMIN, WCAP).astype(int)
    off = np.clip(np.round(cx[idx] - ws / 2.0), 0, W - ws).astype(int)
    return idx, ws, off


def _greedy_f2b(idx, ws, off):
    """Front-to-back (topmost first) order, widest-available-first among
    circles whose later-drawn column-overlapping circles are all emitted."""
    n = len(idx)
    lo, hi = off, off + ws
    done = np.zeros(n, bool)
    order = []
    for _ in range(n):
        best, bestw = -1, -1
        for j in range(n):
            if done[j]:
                continue
            ok = True
            for p in range(n):
                if p == j or done[p]:
                    continue
                if idx[p] > idx[j] and lo[p] < hi[j] and lo[j] < hi[p]:
                    ok = False
                    break
            if ok and ws[j] > bestw:
                bestw, best = ws[j], j
        order.append(best)
        done[best] = True
    return np.array(order, int)


def make_plan(centers, radii):
    """Per-core ordered circle lists + global slot width profile."""
    percore = []
    for core in range(N_CORES):
        idx, ws, off = _core_circles(centers, radii, core)
        o = _greedy_f2b(idx, ws, off)
        percore.append((idx[o], ws[o], off[o]))
    S = max(len(p[0]) for p in percore)
    S = ((S + 7) // 8) * 8
    slotw = np.full(S, WMIN, int)
    for idx, ws, off in percore:
        slotw[:len(ws)] = np.maximum(slotw[:len(ws)], ws)
    return percore, slotw


def _hilo(x):
    hi = x.astype(BF)
    lo = (x - hi.astype(np.float64)).astype(BF)
    return hi, lo


def make_inputs(centers, radii, colors, plan):
    percore, slotw = plan
    S = len(slotw)
    npairs = S // 2
    pairw = [int(slotw[2 * i] + slotw[2 * i + 1]) for i in range(npairs)]
    assert all(pw <= 448 for pw in pairw)
    rhs_len = sum(pairw)
    pair_start = np.concatenate([[0], np.cumsum(pairw)]).astype(int)

    cy = centers[:, 1].astype(np.float64)
    cx = centers[:, 0].astype(np.float64)
    r = radii.astype(np.float64)
    col = colors.astype(np.float64)

    ins = []
    for core in range(N_CORES):
        y0 = ROWS * core
        ids, ws, offs_c = percore[core]
        n = len(ids)
        scal = np.zeros((ROWS, S * 4), np.float32)
        offs = np.zeros((1, S), np.int32)
        lhsT = np.zeros((8, npairs * ROWS), BF)
        rhs = np.zeros((8, rhs_len), BF)
        p = y0 + np.arange(ROWS, dtype=np.float64) + 0.5
        for k in range(n):
            i = ids[k]
            vk = int(slotw[k])
            off = int(np.clip(offs_c[k] + (ws[k] - vk) // 2, 0, W - vk))
            offs[0, k] = off
            al = col[i, 3]
            scal[:, k * 4 + 0] = -al
            scal[:, k * 4 + 1] = al * col[i, 0]
            scal[:, k * 4 + 2] = al * col[i, 1]
            scal[:, k * 4 + 3] = al * col[i, 2]
            j = off + np.arange(vk, dtype=np.float64) + 0.5
            a = r[i] / 2.0 - (p - cy[i]) ** 2 / r[i]
            b = r[i] / 2.0 - (j - cx[i]) ** 2 / r[i]
            ah, alo = _hilo(a)
            bh, blo = _hilo(b)
            pair, half = divmod(k, 2)
            rb = 4 * half
            ls = slice(pair * ROWS, (pair + 1) * ROWS)
            lhsT[rb + 0, ls] = ah
            lhsT[rb + 1, ls] = alo
            lhsT[rb + 2, ls] = 1.0
            lhsT[rb + 3, ls] = 1.0
            c0 = pair_start[pair] + (0 if half == 0 else int(slotw[2 * pair]))
            rs = slice(c0, c0 + vk)
            rhs[rb + 0, rs] = 1.0
            rhs[rb + 1, rs] = 1.0
            rhs[rb + 2, rs] = bh
            rhs[rb + 3, rs] = blo
        ins.append({"scal": scal, "sc16": scal.astype(np.float16),
                    "offs": offs, "lhsT": lhsT, "rhs": rhs})
    return ins


# ------------------------------------------------------------- device build
def build_nc(slotw):
    slotw = [int(v) for v in slotw]
    S = len(slotw)
    npairs = S // 2
    pairw = [slotw[2 * i] + slotw[2 * i + 1] for i in range(npairs)]
    pair_start = [0]
    for pw in pairw:
        pair_start.append(pair_start[-1] + pw)
    rhs_len = pair_start[-1]
    ngroups = (S + 7) // 8

    nc = bacc.Bacc("TRN2", target_bir_lowering=False, debug=False,
                   num_devices=N_CORES)
    scal_d = nc.dram_tensor("scal", [ROWS, S * 4], F32,
                            kind="ExternalInput").ap()
    sc16_d = nc.dram_tensor("sc16", [ROWS, S * 4], F16,
                            kind="ExternalInput").ap()
    offs_d = nc.dram_tensor("offs", [1, S], I32, kind="ExternalInput").ap()
    lhsT_d = nc.dram_tensor("lhsT", [8, npairs * ROWS], BF16,
                            kind="ExternalInput").ap()
    rhs_d = nc.dram_tensor("rhs", [8, rhs_len], BF16,
                           kind="ExternalInput").ap()
    out_d = nc.dram_tensor("out", [ROWS, 4 * W], F16,
                           kind="ExternalOutput").ap()

    with TileContext(nc) as tc:
        T = nc.alloc_sbuf_tensor("T", [ROWS, W], F16).ap()
        CC = nc.alloc_sbuf_tensor("CC", [ROWS, 3 * W], F16).ap()
        AT = nc.alloc_sbuf_tensor("AT", [ROWS, W], F16).ap()
        covr = nc.alloc_sbuf_tensor("covr", [ROWS, 2 * 1792], F16).ap()
        wr = nc.alloc_sbuf_tensor("wr", [ROWS, 8 * WCAP], F16).ap()
        mr = nc.alloc_sbuf_tensor("mr", [ROWS, 4 * 3 * WCAP], F16).ap()
        scal_sb = nc.alloc_sbuf_tensor("scal_sb", [ROWS, S * 4], F32).ap()
        sc16_sb = nc.alloc_sbuf_tensor("sc16_sb", [ROWS, S * 4], F16).ap()
        offs_sb = nc.alloc_sbuf_tensor("offs_sb", [1, S], I32).ap()

        nc.sync.dma_start(scal_sb, scal_d)
        nc.sync.dma_start(sc16_sb, sc16_d)
        nc.sync.dma_start(offs_sb, offs_d)
        nc.vector.memset(T, 1.0)
        nc.vector.memset(CC, 0.0)

        CC3 = CC.rearrange("p (c x) -> p c x", x=W)
        mr3 = mr.rearrange("p (s x) -> p s x", x=WCAP)

        with (
            tc.tile_pool(name="psum", bufs=2, space="PSUM") as psum_pool,
            tc.tile_pool(name="ops", bufs=3) as oppool,
        ):
            pend = None  # (slot k, width, pool-offset) awaiting C adds
            for g in range(ngroups):
                k0 = g * 8
                p0 = k0 // 2
                gw = pair_start[p0 + 4] - pair_start[p0]
                lh_t = oppool.tile([8, 4 * ROWS], BF16, tag="lh")
                rh_t = oppool.tile([8, 1792], BF16, tag="rh")
                nc.sync.dma_start(lh_t, lhsT_d[:, p0 * ROWS:(p0 + 4) * ROWS])
                nc.sync.dma_start(rh_t[:, :gw],
                                  rhs_d[:, pair_start[p0]:pair_start[p0 + 4]])
                pt = psum_pool.tile([ROWS, 4 * 512], F32)
                cbase = (g % 2) * 1792
                rpos = 0
                for i in range(4):
                    pw = pairw[p0 + i]
                    nc.tensor.matmul(
                        pt[:, i * 512:i * 512 + pw],
                        lh_t[:, i * ROWS:(i + 1) * ROWS],
                        rh_t[:, rpos:rpos + pw],
                        start=True, stop=True)
                    nc.scalar.activation(
                        covr[:, cbase + rpos:cbase + rpos + pw],
                        pt[:, i * 512:i * 512 + pw], AF.Sigmoid)
                    rpos += pw

                # offsets for the group on Pool (w) and DVE (T chain, C adds)
                gregs = [nc.gpsimd.alloc_register(f"off_{k0}_{i}")
                         for i in range(8)]
                nc.gpsimd.reg_load(gregs, offs_sb[0:1, k0:k0 + 8])
                goff = [nc.gpsimd.snap(gregs[j], donate=True, min_val=0,
                                       max_val=W - slotw[k0 + j])
                        for j in range(8)]
                vregs = [nc.vector.alloc_register(f"voff_{k0}_{i}")
                         for i in range(8)]
                nc.vector.reg_load(vregs, offs_sb[0:1, k0:k0 + 8])
                voff = [nc.vector.snap(vregs[j], donate=True, min_val=0,
                                       max_val=W - slotw[k0 + j])
                        for j in range(8)]

                rpos = 0
                for j in range(8):
                    k = k0 + j
                    vk = slotw[k]
                    cov = covr[:, cbase + rpos:cbase + rpos + vk]
                    rpos += vk
                    wv = wr[:, (k % 8) * WCAP:(k % 8) * WCAP + vk]
                    tw = T[:, bass.ds(goff[j], vk)]
                    # Pool: w = T*cov  (Pool's only slot op)
                    nc.gpsimd.tensor_tensor(wv, tw, cov, OP.mult)
                    # DVE: T = (-a)*w + T  (chain-critical, emit first)
                    twv = T[:, bass.ds(voff[j], vk)]
                    nc.vector.scalar_tensor_tensor(
                        twv, wv, scal_sb[:, k * 4:k * 4 + 1], twv,
                        OP.mult, OP.add)
                    if pend is not None:
                        kp, vp, offp = pend
                        qv = CC3[:, :, bass.ds(offp, vp)]
                        mp = mr3[:, (kp % 4) * 3:(kp % 4) * 3 + 3, :vp]
                        nc.vector.tensor_tensor(qv, qv, mp, OP.add)
                    # ACT: m_B = (a colB) * w
                    mbB = ((k % 4) * 3 + 2) * WCAP
                    nc.scalar.activation(
                        mr[:, mbB:mbB + vk], wv, AF.Copy,
                        scale=scal_sb[:, k * 4 + 3:k * 4 + 4])
                    # DVE broadcast premultiply: m2[ch] = (a col_ch) * w
                    w3 = wv.rearrange("p (c x) -> p c x", c=1)
                    sv2 = sc16_sb[:, k * 4 + 1:k * 4 + 3].rearrange(
                        "p (c x) -> p c x", x=1)
                    b0, b1 = bass.broadcast_tensor_aps(w3, sv2)
                    nc.vector.tensor_tensor(
                        mr3[:, (k % 4) * 3:(k % 4) * 3 + 2, :vk],
                        b0, b1, OP.mult)
                    pend = (k, vk, voff[j])

            if pend is not None:
                kp, vp, offp = pend
                qv = CC3[:, :, bass.ds(offp, vp)]
                mp = mr3[:, (kp % 4) * 3:(kp % 4) * 3 + 3, :vp]
                nc.vector.tensor_tensor(qv, qv, mp, OP.add)
                pend = None

        # A = 1 - T, then plane DMAs
        nc.vector.tensor_scalar(AT, T, -1.0, 1.0, OP.mult, OP.add)
        nc.sync.dma_start(out_d[:, 0:3 * W], CC)
        nc.sync.dma_start(out_d[:, 3 * W:4 * W], AT)

    nc.compile()
    return nc


_CACHE = {}


def _get_nc(slotw):
    key = tuple(int(v) for v in slotw)
    if key not in _CACHE:
        _CACHE[key] = build_nc(slotw)
    return _CACHE[key]


def kernel(centers, radii, colors):
    centers = np.asarray(centers, np.float32)
    radii = np.asarray(radii, np.float32)
    colors = np.asarray(colors, np.float32)

    plan = make_plan(centers, radii)
    nc = _get_nc(plan[1])
    ins = make_inputs(centers, radii, colors, plan)
    res = bass_utils.run_bass_kernel_spmd(nc, ins, list(range(N_CORES)),
                                          trace=False)
    out = np.empty((H, W, 4), np.float32)
    for c in range(N_CORES):
        planes = res.results[c]["out"].astype(np.float32)  # [128, 4*W]
        for ch in range(4):
            out[c * ROWS:(c + 1) * ROWS, :, ch] = planes[:, ch * W:(ch + 1) * W]
    return out


# revision 16
# speedup vs baseline: 1.6991x; 1.0593x over previous
"""DiffVG-style circle renderer on 8 Trainium2 NeuronCores.

Strategy: shard the 1024x1024 image by rows (128 rows per core). Each core
composites the circles whose vertical span intersects its row band,
front-to-back with transmittance T:

    cov = sigmoid(r - d^2/r)          ~= sigmoid(2(r - d)) near the edge
    w   = T * cov                      (w ring, fp16)
    T  *= (1 - a*cov)                  (mtau ring premultiplied on DVE)
    C_ch += (a*col_ch) * w             (premultiplied m_ch on DVE)

Front-to-back order is relaxed: circles whose column windows don't overlap
commute, so each core emits a width-descending order compatible with the
z partial order. Slot k's window width is the max over cores of the k-th
emitted circle width (compile-time constant); offsets are runtime data.

Engine split (all dynamic-window ops as cheap Pool tensor_tensor):
  PE     z = (r^2 - d^2)/r outer-sum; two circles per K=8 matmul
         (bf16 hi/lo split operands), bias folded in -> no sqrt pass
  ACT    per-pair sigmoid PSUM -> fp16 cov ring
  DVE    static-ring tensor_scalar premultiplies: mtau = 1 - a*cov,
         m_ch = (a col_ch) * w
  Pool   w = T*cov, T *= mtau, C_ch += m_ch  on dynamic windows
State T, CR, CG, CB are fp16 planes; output = 4 fp16 planes DMA'd out,
assembled/converted to f32 on host.
"""

import sys

if "/opt/trn_rl_repo" not in sys.path:
    sys.path.insert(0, "/opt/trn_rl_repo")

import numpy as np
import ml_dtypes

import concourse.bass as bass
import concourse.bacc as bacc
import concourse.mybir as mybir
from concourse.tile import TileContext
from concourse import bass_utils

H = 1024
W = 1024
ROWS = 128
N_CORES = 8
MARGIN = 5.0
ROUND = 8
WMIN = 24
WCAP = 224
F32 = mybir.dt.float32
F16 = mybir.dt.float16
BF16 = mybir.dt.bfloat16
I32 = mybir.dt.int32
AF = mybir.ActivationFunctionType
OP = mybir.AluOpType
BF = ml_dtypes.bfloat16


# ---------------------------------------------------------------- host plan
def _core_circles(centers, radii, core):
    """Kept circle indices + cap-clipped rounded widths + offsets."""
    y0 = ROWS * core
    cy = centers[:, 1].astype(np.float64)
    cx = centers[:, 0].astype(np.float64)
    r = radii.astype(np.float64)
    keep = (cy + r + MARGIN >= y0 + 0.5) & (cy - r - MARGIN <= y0 + ROWS - 0.5)
    idx = np.where(keep)[0]
    dymin = np.maximum(0.0, np.maximum(y0 + 0.5 - cy[idx],
                                       cy[idx] - (y0 + ROWS - 0.5)))
    rm = r[idx] + MARGIN
    halfw = np.sqrt(np.maximum(rm * rm - dymin * dymin, 4.0))
    ws = np.clip(np.ceil(2.0 * halfw / ROUND) * ROUND, WMIN, WCAP).astype(int)
    off = np.clip(np.round(cx[idx] - ws / 2.0), 0, W - ws).astype(int)
    return idx, ws, off


def _greedy_f2b(idx, ws, off):
    """Front-to-back (topmost first) order, widest-available-first among
    circles whose later-drawn column-overlapping circles are all emitted."""
    n = len(idx)
    lo, hi = off, off + ws
    done = np.zeros(n, bool)
    order = []
    for _ in range(n):
        best, bestw = -1, -1
        for j in range(n):
            if done[j]:
                continue
            ok = True
            for p in range(n):
                if p == j or done[p]:
                    continue
                if idx[p] > idx[j] and lo[p] < hi[j] and lo[j] < hi[p]:
                    ok = False
                    break
            if ok and ws[j] > bestw:
                bestw, best = ws[j], j
        order.append(best)
        done[best] = True
    return np.array(order, int)


def make_plan(centers, radii):
    """Per-core ordered circle lists + global slot width profile."""
    percore = []
    for core in range(N_CORES):
        idx, ws, off = _core_circles(centers, radii, core)
        o = _greedy_f2b(idx, ws, off)
        percore.append((idx[o], ws[o], off[o]))
    S = max(len(p[0]) for p in percore)
    S = ((S + 7) // 8) * 8
    slotw = np.full(S, WMIN, int)
    for idx, ws, off in percore:
        slotw[:len(ws)] = np.maximum(slotw[:len(ws)], ws)
    return percore, slotw


def _hilo(x):
    hi = x.astype(BF)
    lo = (x - hi.astype(np.float64)).astype(BF)
    return hi, lo


def make_inputs(centers, radii, colors, plan):
    percore, slotw = plan
    S = len(slotw)
    npairs = S // 2
    pairw = [int(slotw[2 * i] + slotw[2 * i + 1]) for i in range(npairs)]
    assert all(pw <= 448 for pw in pairw)
    rhs_len = sum(pairw)
    pair_start = np.concatenate([[0], np.cumsum(pairw)]).astype(int)

    cy = centers[:, 1].astype(np.float64)
    cx = centers[:, 0].astype(np.float64)
    r = radii.astype(np.float64)
    col = colors.astype(np.float64)

    ins = []
    for core in range(N_CORES):
        y0 = ROWS * core
        ids, ws, offs_c = percore[core]
        n = len(ids)
        scal = np.zeros((ROWS, S * 4), np.float32)
        offs = np.zeros((1, S), np.int32)
        lhsT = np.zeros((8, npairs * ROWS), BF)
        rhs = np.zeros((8, rhs_len), BF)
        p = y0 + np.arange(ROWS, dtype=np.float64) + 0.5
        for k in range(n):
            i = ids[k]
            vk = int(slotw[k])
            off = int(np.clip(offs_c[k] + (ws[k] - vk) // 2, 0, W - vk))
            offs[0, k] = off
            al = col[i, 3]
            scal[:, k * 4 + 0] = -al
            scal[:, k * 4 + 1] = al * col[i, 0]
            scal[:, k * 4 + 2] = al * col[i, 1]
            scal[:, k * 4 + 3] = al * col[i, 2]
            j = off + np.arange(vk, dtype=np.float64) + 0.5
            a = r[i] / 2.0 - (p - cy[i]) ** 2 / r[i]
            b = r[i] / 2.0 - (j - cx[i]) ** 2 / r[i]
            ah, alo = _hilo(a)
            bh, blo = _hilo(b)
            pair, half = divmod(k, 2)
            rb = 4 * half
            ls = slice(pair * ROWS, (pair + 1) * ROWS)
            lhsT[rb + 0, ls] = ah
            lhsT[rb + 1, ls] = alo
            lhsT[rb + 2, ls] = 1.0
            lhsT[rb + 3, ls] = 1.0
            c0 = pair_start[pair] + (0 if half == 0 else int(slotw[2 * pair]))
            rs = slice(c0, c0 + vk)
            rhs[rb + 0, rs] = 1.0
            rhs[rb + 1, rs] = 1.0
            rhs[rb + 2, rs] = bh
            rhs[rb + 3, rs] = blo
        ins.append({"scal": scal, "sc16": scal.astype(np.float16),
                    "offs": offs, "lhsT": lhsT, "rhs": rhs})
    return ins


# ------------------------------------------------------------- device build
def build_nc(slotw):
    slotw = [int(v) for v in slotw]
    S = len(slotw)
    npairs = S // 2
    pairw = [slotw[2 * i] + slotw[2 * i + 1] for i in range(npairs)]
    pair_start = [0]
    for pw in pairw:
        pair_start.append(pair_start[-1] + pw)
    rhs_len = pair_start[-1]
    ngroups = (S + 7) // 8

    nc = bacc.Bacc("TRN2", target_bir_lowering=False, debug=False,
                   num_devices=N_CORES)
    scal_d = nc.dram_tensor("scal", [ROWS, S * 4], F32,
                            kind="ExternalInput").ap()
    sc16_d = nc.dram_tensor("sc16", [ROWS, S * 4], F16,
                            kind="ExternalInput").ap()
    offs_d = nc.dram_tensor("offs", [1, S], I32, kind="ExternalInput").ap()
    lhsT_d = nc.dram_tensor("lhsT", [8, npairs * ROWS], BF16,
                            kind="ExternalInput").ap()
    rhs_d = nc.dram_tensor("rhs", [8, rhs_len], BF16,
                           kind="ExternalInput").ap()
    out_d = nc.dram_tensor("out", [ROWS, 4 * W], F16,
                           kind="ExternalOutput").ap()

    with TileContext(nc) as tc:
        T = nc.alloc_sbuf_tensor("T", [ROWS, W], F16).ap()
        CC = nc.alloc_sbuf_tensor("CC", [ROWS, 3 * W], F16).ap()
        AT = nc.alloc_sbuf_tensor("AT", [ROWS, W], F16).ap()
        covr = nc.alloc_sbuf_tensor("covr", [ROWS, 2 * 1792], F16).ap()
        wr = nc.alloc_sbuf_tensor("wr", [ROWS, 8 * WCAP], F16).ap()
        mr = nc.alloc_sbuf_tensor("mr", [ROWS, 4 * 3 * WCAP], F16).ap()
        scal_sb = nc.alloc_sbuf_tensor("scal_sb", [ROWS, S * 4], F32).ap()
        sc16_sb = nc.alloc_sbuf_tensor("sc16_sb", [ROWS, S * 4], F16).ap()
        offs_sb = nc.alloc_sbuf_tensor("offs_sb", [1, S], I32).ap()

        nc.sync.dma_start(scal_sb, scal_d)
        nc.sync.dma_start(sc16_sb, sc16_d)
        nc.sync.dma_start(offs_sb, offs_d)
        nc.vector.memset(T, 1.0)
        nc.vector.memset(CC, 0.0)

        CC3 = CC.rearrange("p (c x) -> p c x", x=W)
        mr3 = mr.rearrange("p (s x) -> p s x", x=WCAP)

        with (
            tc.tile_pool(name="psum", bufs=2, space="PSUM") as psum_pool,
            tc.tile_pool(name="ops", bufs=3) as oppool,
        ):
            pend = None  # (slot k, width, pool-offset) awaiting C adds
            for g in range(ngroups):
                k0 = g * 8
                p0 = k0 // 2
                gw = pair_start[p0 + 4] - pair_start[p0]
                lh_t = oppool.tile([8, 4 * ROWS], BF16, tag="lh")
                rh_t = oppool.tile([8, 1792], BF16, tag="rh")
                nc.sync.dma_start(lh_t, lhsT_d[:, p0 * ROWS:(p0 + 4) * ROWS])
                nc.sync.dma_start(rh_t[:, :gw],
                                  rhs_d[:, pair_start[p0]:pair_start[p0 + 4]])
                pt = psum_pool.tile([ROWS, 4 * 512], F32)
                cbase = (g % 2) * 1792
                rpos = 0
                for i in range(4):
                    pw = pairw[p0 + i]
                    nc.tensor.matmul(
                        pt[:, i * 512:i * 512 + pw],
                        lh_t[:, i * ROWS:(i + 1) * ROWS],
                        rh_t[:, rpos:rpos + pw],
                        start=True, stop=True)
                    nc.scalar.activation(
                        covr[:, cbase + rpos:cbase + rpos + pw],
                        pt[:, i * 512:i * 512 + pw], AF.Sigmoid)
                    rpos += pw

                # offsets for the group on Pool (w) and DVE (T chain, C adds)
                gregs = [nc.gpsimd.alloc_register(f"off_{k0}_{i}")
                         for i in range(8)]
                nc.gpsimd.reg_load(gregs, offs_sb[0:1, k0:k0 + 8])
                goff = [nc.gpsimd.snap(gregs[j], donate=True, min_val=0,
                                       max_val=W - slotw[k0 + j])
                        for j in range(8)]
                vregs = [nc.vector.alloc_register(f"voff_{k0}_{i}")
                         for i in range(8)]
                nc.vector.reg_load(vregs, offs_sb[0:1, k0:k0 + 8])
                voff = [nc.vector.snap(vregs[j], donate=True, min_val=0,
                                       max_val=W - slotw[k0 + j])
                        for j in range(8)]

                rpos = 0
                for j in range(8):
                    k = k0 + j
                    vk = slotw[k]
                    cov = covr[:, cbase + rpos:cbase + rpos + vk]
                    rpos += vk
                    wv = wr[:, (k % 8) * WCAP:(k % 8) * WCAP + vk]
                    tw = T[:, bass.ds(goff[j], vk)]
                    # Pool: w = T*cov  (Pool's only slot op)
                    nc.gpsimd.tensor_tensor(wv, tw, cov, OP.mult)
                    # DVE: T = (-a)*w + T  (chain-critical, emit first)
                    twv = T[:, bass.ds(voff[j], vk)]
                    nc.vector.scalar_tensor_tensor(
                        twv, wv, scal_sb[:, k * 4:k * 4 + 1], twv,
                        OP.mult, OP.add)
                    if pend is not None:
                        kp, vp, offp = pend
                        qv = CC3[:, :, bass.ds(offp, vp)]
                        mp = mr3[:, (kp % 4) * 3:(kp % 4) * 3 + 3, :vp]
                        nc.vector.tensor_tensor(qv, qv, mp, OP.add)
                    # ACT: m_G, m_B = (a col_ch) * w
                    mbG = ((k % 4) * 3 + 1) * WCAP
                    nc.scalar.activation(
                        mr[:, mbG:mbG + vk], wv, AF.Copy,
                        scale=scal_sb[:, k * 4 + 2:k * 4 + 3])
                    mbB = ((k % 4) * 3 + 2) * WCAP
                    nc.scalar.activation(
                        mr[:, mbB:mbB + vk], wv, AF.Copy,
                        scale=scal_sb[:, k * 4 + 3:k * 4 + 4])
                    # DVE: m_R = (a colR) * w  (tensor_scalar, 4x-eligible)
                    mbR = (k % 4) * 3 * WCAP
                    nc.vector.tensor_scalar(
                        mr[:, mbR:mbR + vk], wv,
                        scal_sb[:, k * 4 + 1:k * 4 + 2], 0.0,
                        OP.mult, OP.add)
                    pend = (k, vk, voff[j])

            if pend is not None:
                kp, vp, offp = pend
                qv = CC3[:, :, bass.ds(offp, vp)]
                mp = mr3[:, (kp % 4) * 3:(kp % 4) * 3 + 3, :vp]
                nc.vector.tensor_tensor(qv, qv, mp, OP.add)
                pend = None

        # A = 1 - T, then plane DMAs
        nc.vector.tensor_scalar(AT, T, -1.0, 1.0, OP.mult, OP.add)
        nc.sync.dma_start(out_d[:, 0:3 * W], CC)
        nc.sync.dma_start(out_d[:, 3 * W:4 * W], AT)

    nc.compile()
    return nc


_CACHE = {}


def _get_nc(slotw):
    key = tuple(int(v) for v in slotw)
    if key not in _CACHE:
        _CACHE[key] = build_nc(slotw)
    return _CACHE[key]


def kernel(centers, radii, colors):
    centers = np.asarray(centers, np.float32)
    radii = np.asarray(radii, np.float32)
    colors = np.asarray(colors, np.float32)

    plan = make_plan(centers, radii)
    nc = _get_nc(plan[1])
    ins = make_inputs(centers, radii, colors, plan)
    res = bass_utils.run_bass_kernel_spmd(nc, ins, list(range(N_CORES)),
                                          trace=False)
    out = np.empty((H, W, 4), np.float32)
    for c in range(N_CORES):
        planes = res.results[c]["out"].astype(np.float32)  # [128, 4*W]
        for ch in range(4):
            out[c * ROWS:(c + 1) * ROWS, :, ch] = planes[:, ch * W:(ch + 1) * W]
    return out


# revision 18
# speedup vs baseline: 1.7340x; 1.0206x over previous
"""DiffVG-style circle renderer on 8 Trainium2 NeuronCores.

Strategy: shard the 1024x1024 image by rows (128 rows per core). Each core
composites the circles whose vertical span intersects its row band,
front-to-back with transmittance T:

    cov = sigmoid(r - d^2/r)          ~= sigmoid(2(r - d)) near the edge
    w   = T * cov                      (w ring, fp16)
    T  *= (1 - a*cov)                  (mtau ring premultiplied on DVE)
    C_ch += (a*col_ch) * w             (premultiplied m_ch on DVE)

Front-to-back order is relaxed: circles whose column windows don't overlap
commute, so each core emits a width-descending order compatible with the
z partial order. Slot k's window width is the max over cores of the k-th
emitted circle width (compile-time constant); offsets are runtime data.

Engine split (all dynamic-window ops as cheap Pool tensor_tensor):
  PE     z = (r^2 - d^2)/r outer-sum; two circles per K=8 matmul
         (bf16 hi/lo split operands), bias folded in -> no sqrt pass
  ACT    per-pair sigmoid PSUM -> fp16 cov ring
  DVE    static-ring tensor_scalar premultiplies: mtau = 1 - a*cov,
         m_ch = (a col_ch) * w
  Pool   w = T*cov, T *= mtau, C_ch += m_ch  on dynamic windows
State T, CR, CG, CB are fp16 planes; output = 4 fp16 planes DMA'd out,
assembled/converted to f32 on host.
"""

import sys

if "/opt/trn_rl_repo" not in sys.path:
    sys.path.insert(0, "/opt/trn_rl_repo")

import numpy as np
import ml_dtypes

import concourse.bass as bass
import concourse.bacc as bacc
import concourse.mybir as mybir
from concourse.tile import TileContext
from concourse import bass_utils

H = 1024
W = 1024
ROWS = 128
N_CORES = 8
MARGIN = 5.0
ROUND = 8
WMIN = 24
WCAP = 224
F32 = mybir.dt.float32
F16 = mybir.dt.float16
BF16 = mybir.dt.bfloat16
I32 = mybir.dt.int32
AF = mybir.ActivationFunctionType
OP = mybir.AluOpType
BF = ml_dtypes.bfloat16


# ---------------------------------------------------------------- host plan
def _core_circles(centers, radii, core):
    """Kept circle indices + cap-clipped rounded widths + offsets."""
    y0 = ROWS * core
    cy = centers[:, 1].astype(np.float64)
    cx = centers[:, 0].astype(np.float64)
    r = radii.astype(np.float64)
    keep = (cy + r + MARGIN >= y0 + 0.5) & (cy - r - MARGIN <= y0 + ROWS - 0.5)
    idx = np.where(keep)[0]
    dymin = np.maximum(0.0, np.maximum(y0 + 0.5 - cy[idx],
                                       cy[idx] - (y0 + ROWS - 0.5)))
    rm = r[idx] + MARGIN
    halfw = np.sqrt(np.maximum(rm * rm - dymin * dymin, 4.0))
    ws = np.clip(np.ceil(2.0 * halfw / ROUND) * ROUND, WMIN, WCAP).astype(int)
    off = np.clip(np.round(cx[idx] - ws / 2.0), 0, W - ws).astype(int)
    return idx, ws, off


def _greedy_f2b(idx, ws, off):
    """Front-to-back (topmost first) order, widest-available-first among
    circles whose later-drawn column-overlapping circles are all emitted."""
    n = len(idx)
    lo, hi = off, off + ws
    done = np.zeros(n, bool)
    order = []
    for _ in range(n):
        best, bestw = -1, -1
        for j in range(n):
            if done[j]:
                continue
            ok = True
            for p in range(n):
                if p == j or done[p]:
                    continue
                if idx[p] > idx[j] and lo[p] < hi[j] and lo[j] < hi[p]:
                    ok = False
                    break
            if ok and ws[j] > bestw:
                bestw, best = ws[j], j
        order.append(best)
        done[best] = True
    return np.array(order, int)


def make_plan(centers, radii):
    """Per-core ordered circle lists + global slot width profile."""
    percore = []
    for core in range(N_CORES):
        idx, ws, off = _core_circles(centers, radii, core)
        o = _greedy_f2b(idx, ws, off)
        percore.append((idx[o], ws[o], off[o]))
    S = max(len(p[0]) for p in percore)
    S = ((S + 7) // 8) * 8
    slotw = np.full(S, WMIN, int)
    for idx, ws, off in percore:
        slotw[:len(ws)] = np.maximum(slotw[:len(ws)], ws)
    return percore, slotw


def _hilo(x):
    hi = x.astype(BF)
    lo = (x - hi.astype(np.float64)).astype(BF)
    return hi, lo


def make_inputs(centers, radii, colors, plan):
    percore, slotw = plan
    S = len(slotw)
    npairs = S // 2
    pairw = [int(slotw[2 * i] + slotw[2 * i + 1]) for i in range(npairs)]
    assert all(pw <= 448 for pw in pairw)
    rhs_len = sum(pairw)
    pair_start = np.concatenate([[0], np.cumsum(pairw)]).astype(int)

    cy = centers[:, 1].astype(np.float64)
    cx = centers[:, 0].astype(np.float64)
    r = radii.astype(np.float64)
    col = colors.astype(np.float64)

    ins = []
    for core in range(N_CORES):
        y0 = ROWS * core
        ids, ws, offs_c = percore[core]
        n = len(ids)
        scal = np.zeros((ROWS, S * 4), np.float32)
        offs = np.zeros((1, S), np.int32)
        lhsT = np.zeros((8, npairs * ROWS), BF)
        rhs = np.zeros((8, rhs_len), BF)
        p = y0 + np.arange(ROWS, dtype=np.float64) + 0.5
        for k in range(n):
            i = ids[k]
            vk = int(slotw[k])
            off = int(np.clip(offs_c[k] + (ws[k] - vk) // 2, 0, W - vk))
            offs[0, k] = off
            al = col[i, 3]
            scal[:, k * 4 + 0] = -al
            scal[:, k * 4 + 1] = al * col[i, 0]
            scal[:, k * 4 + 2] = al * col[i, 1]
            scal[:, k * 4 + 3] = al * col[i, 2]
            j = off + np.arange(vk, dtype=np.float64) + 0.5
            a = r[i] / 2.0 - (p - cy[i]) ** 2 / r[i]
            b = r[i] / 2.0 - (j - cx[i]) ** 2 / r[i]
            ah, alo = _hilo(a)
            bh, blo = _hilo(b)
            pair, half = divmod(k, 2)
            rb = 4 * half
            ls = slice(pair * ROWS, (pair + 1) * ROWS)
            lhsT[rb + 0, ls] = ah
            lhsT[rb + 1, ls] = alo
            lhsT[rb + 2, ls] = 1.0
            lhsT[rb + 3, ls] = 1.0
            c0 = pair_start[pair] + (0 if half == 0 else int(slotw[2 * pair]))
            rs = slice(c0, c0 + vk)
            rhs[rb + 0, rs] = 1.0
            rhs[rb + 1, rs] = 1.0
            rhs[rb + 2, rs] = bh
            rhs[rb + 3, rs] = blo
        ins.append({"scal": scal, "offs": offs, "lhsT": lhsT, "rhs": rhs})
    return ins


# ------------------------------------------------------------- device build
def build_nc(slotw):
    slotw = [int(v) for v in slotw]
    S = len(slotw)
    npairs = S // 2
    pairw = [slotw[2 * i] + slotw[2 * i + 1] for i in range(npairs)]
    pair_start = [0]
    for pw in pairw:
        pair_start.append(pair_start[-1] + pw)
    rhs_len = pair_start[-1]
    ngroups = (S + 7) // 8

    nc = bacc.Bacc("TRN2", target_bir_lowering=False, debug=False,
                   num_devices=N_CORES)
    scal_d = nc.dram_tensor("scal", [ROWS, S * 4], F32,
                            kind="ExternalInput").ap()
    offs_d = nc.dram_tensor("offs", [1, S], I32, kind="ExternalInput").ap()
    lhsT_d = nc.dram_tensor("lhsT", [8, npairs * ROWS], BF16,
                            kind="ExternalInput").ap()
    rhs_d = nc.dram_tensor("rhs", [8, rhs_len], BF16,
                           kind="ExternalInput").ap()
    out_d = nc.dram_tensor("out", [ROWS, 4 * W], F16,
                           kind="ExternalOutput").ap()

    with TileContext(nc) as tc:
        T = nc.alloc_sbuf_tensor("T", [ROWS, W], F16).ap()
        CC = nc.alloc_sbuf_tensor("CC", [ROWS, 3 * W], F16).ap()
        AT = nc.alloc_sbuf_tensor("AT", [ROWS, W], F16).ap()
        covr = nc.alloc_sbuf_tensor("covr", [ROWS, 2 * 1792], F16).ap()
        wr = nc.alloc_sbuf_tensor("wr", [ROWS, 8 * WCAP], F16).ap()
        mr = nc.alloc_sbuf_tensor("mr", [ROWS, 4 * 3 * WCAP], F16).ap()
        scal_sb = nc.alloc_sbuf_tensor("scal_sb", [ROWS, S * 4], F32).ap()
        offs_sb = nc.alloc_sbuf_tensor("offs_sb", [1, S], I32).ap()

        nc.sync.dma_start(scal_sb, scal_d)
        nc.sync.dma_start(offs_sb, offs_d)
        nc.vector.memset(T, 1.0)
        nc.gpsimd.memset(CC, 0.0)

        CC3 = CC.rearrange("p (c x) -> p c x", x=W)
        mr3 = mr.rearrange("p (s x) -> p s x", x=WCAP)

        with (
            tc.tile_pool(name="psum", bufs=2, space="PSUM") as psum_pool,
            tc.tile_pool(name="ops", bufs=3) as oppool,
        ):
            pend = None  # (slot k, width, pool-offset) awaiting C adds
            for g in range(ngroups):
                k0 = g * 8
                p0 = k0 // 2
                gw = pair_start[p0 + 4] - pair_start[p0]
                lh_t = oppool.tile([8, 4 * ROWS], BF16, tag="lh")
                rh_t = oppool.tile([8, 1792], BF16, tag="rh")
                nc.sync.dma_start(lh_t, lhsT_d[:, p0 * ROWS:(p0 + 4) * ROWS])
                nc.sync.dma_start(rh_t[:, :gw],
                                  rhs_d[:, pair_start[p0]:pair_start[p0 + 4]])
                pt = psum_pool.tile([ROWS, 4 * 512], F32)
                cbase = (g % 2) * 1792
                rpos = 0
                for i in range(4):
                    pw = pairw[p0 + i]
                    nc.tensor.matmul(
                        pt[:, i * 512:i * 512 + pw],
                        lh_t[:, i * ROWS:(i + 1) * ROWS],
                        rh_t[:, rpos:rpos + pw],
                        start=True, stop=True)
                    nc.scalar.activation(
                        covr[:, cbase + rpos:cbase + rpos + pw],
                        pt[:, i * 512:i * 512 + pw], AF.Sigmoid)
                    rpos += pw

                # offsets for the group on Pool (w) and DVE (T chain, C adds)
                gregs = [nc.gpsimd.alloc_register(f"off_{k0}_{i}")
                         for i in range(8)]
                nc.gpsimd.reg_load(gregs, offs_sb[0:1, k0:k0 + 8])
                goff = [nc.gpsimd.snap(gregs[j], donate=True, min_val=0,
                                       max_val=W - slotw[k0 + j])
                        for j in range(8)]
                vregs = [nc.vector.alloc_register(f"voff_{k0}_{i}")
                         for i in range(8)]
                nc.vector.reg_load(vregs, offs_sb[0:1, k0:k0 + 8])
                voff = [nc.vector.snap(vregs[j], donate=True, min_val=0,
                                       max_val=W - slotw[k0 + j])
                        for j in range(8)]

                rpos = 0
                for j in range(8):
                    k = k0 + j
                    vk = slotw[k]
                    cov = covr[:, cbase + rpos:cbase + rpos + vk]
                    rpos += vk
                    wv = wr[:, (k % 8) * WCAP:(k % 8) * WCAP + vk]
                    tw = T[:, bass.ds(goff[j], vk)]
                    # Pool: w = T*cov  (Pool's only slot op)
                    nc.gpsimd.tensor_tensor(wv, tw, cov, OP.mult)
                    # DVE: T = (-a)*w + T  (chain-critical, emit first)
                    twv = T[:, bass.ds(voff[j], vk)]
                    nc.vector.scalar_tensor_tensor(
                        twv, wv, scal_sb[:, k * 4:k * 4 + 1], twv,
                        OP.mult, OP.add)
                    if pend is not None:
                        kp, vp, offp = pend
                        qv = CC3[:, :, bass.ds(offp, vp)]
                        mp = mr3[:, (kp % 4) * 3:(kp % 4) * 3 + 3, :vp]
                        nc.vector.tensor_tensor(qv, qv, mp, OP.add)
                    # Pool: m_G = (a colG) * w ; ACT: m_B = (a colB) * w
                    mbG = ((k % 4) * 3 + 1) * WCAP
                    nc.gpsimd.tensor_scalar(
                        mr[:, mbG:mbG + vk], wv,
                        scal_sb[:, k * 4 + 2:k * 4 + 3], 0.0,
                        OP.mult, OP.add)
                    mbB = ((k % 4) * 3 + 2) * WCAP
                    nc.scalar.activation(
                        mr[:, mbB:mbB + vk], wv, AF.Copy,
                        scale=scal_sb[:, k * 4 + 3:k * 4 + 4])
                    # DVE: m_R = (a colR) * w  (tensor_scalar, 4x-eligible)
                    mbR = (k % 4) * 3 * WCAP
                    nc.vector.tensor_scalar(
                        mr[:, mbR:mbR + vk], wv,
                        scal_sb[:, k * 4 + 1:k * 4 + 2], 0.0,
                        OP.mult, OP.add)
                    pend = (k, vk, voff[j])

            if pend is not None:
                kp, vp, offp = pend
                qv = CC3[:, :, bass.ds(offp, vp)]
                mp = mr3[:, (kp % 4) * 3:(kp % 4) * 3 + 3, :vp]
                nc.vector.tensor_tensor(qv, qv, mp, OP.add)
                pend = None

        # A = 1 - T, then plane DMAs
        nc.vector.tensor_scalar(AT, T, -1.0, 1.0, OP.mult, OP.add)
        nc.sync.dma_start(out_d[:, 0:3 * W], CC)
        nc.sync.dma_start(out_d[:, 3 * W:4 * W], AT)

    nc.compile()
    return nc


_CACHE = {}


def _get_nc(slotw):
    key = tuple(int(v) for v in slotw)
    if key not in _CACHE:
        _CACHE[key] = build_nc(slotw)
    return _CACHE[key]


def kernel(centers, radii, colors):
    centers = np.asarray(centers, np.float32)
    radii = np.asarray(radii, np.float32)
    colors = np.asarray(colors, np.float32)

    plan = make_plan(centers, radii)
    nc = _get_nc(plan[1])
    ins = make_inputs(centers, radii, colors, plan)
    res = bass_utils.run_bass_kernel_spmd(nc, ins, list(range(N_CORES)),
                                          trace=False)
    out = np.empty((H, W, 4), np.float32)
    for c in range(N_CORES):
        planes = res.results[c]["out"].astype(np.float32)  # [128, 4*W]
        for ch in range(4):
            out[c * ROWS:(c + 1) * ROWS, :, ch] = planes[:, ch * W:(ch + 1) * W]
    return out
